# revision 5
# baseline (speedup 1.0000x reference)
"""Trainium2 Bass kernel for nn_Boundary_Enchance (dense_cnn), v2.

Pure data parallel: core i processes batch image i.  Compute is fp8-e4m3
DoubleRow on the PE for the fuse 1x1, the 3x3 conv, and the merged
mask+boundary head (validated rel-err 0.004 on host); the final 1->16
expansion stays bf16 (fp8 weights there cost ~2% systematic error).

Layout tricks:
  - XF tiles [128, 1028] fp8: cols 0..513 = x (8 rows x 16ch, image cols
    -1..512, host-packed), cols 514..1027 = F = relu(fuse(y)) written by the
    fuse evacuation.  The 3x3 conv is THREE DoubleRow matmuls (one per dx
    tap): the DR pair (stride 514) contracts x and F simultaneously, K_eff
    = 256 = 8 rows x 32 ch.  dy taps ride the row-Toeplitz lhsT; dx taps
    are rhs base-column shifts.
  - fuse: one DR matmul per 8-row tile, pair (j, j+2) with zero second
    weights; bias folded into the evacuation tensor_scalar (per-tile bias
    column variants handle the image edges).
  - mask+boundary (LBM): per strip one DR matmul with M=128 accumulating 8
    strips into ONE psum bank (strip g -> mask logits at partitions 6g+i,
    boundary at 64+6g+i, zeros elsewhere).  Tail nonlinearities run ONCE
    per 8 strips: sigmoid (with per-partition bias = head biases), add,
    min -> sv group tile [48, 512].
  - final expansion per strip: either an LC bf16 matmul + psum evacuation,
    or a replicating SBUF DMA + per-partition affine (x cv_w + cv_b) on
    the DVE at 4x rate ("REP path") -- mixed to balance PE/DMA/engines.
"""

import numpy as np
import ml_dtypes

BF16 = ml_dtypes.bfloat16
F8 = ml_dtypes.float8_e4m3

H = 512
W = 512
SB = 6
NT = (H + SB - 1) // SB          # 86 strips
XW = 514                          # x / F block width (image cols -1..512)
PW = 2 * XW                       # XF tile width
FW = 514                          # fcc slot width (512 + 2 pad)
NG = (NT + 7) // 8                # strip groups of 8

RING_YH = 16
TAIL_LAG = 8                      # tails lag fronts in phase B
N_WARM = 40
CHAIN_POOL = False


REP_FRAC = 3                      # u % REP_FRAC == 1 -> REP; 0 disables
CONV_DEPTH = 2                    # strips per conv psum tile group
CHAIN_LAG = 3
FIN_LAG = 8
ACT_W = 1.0


REP_END = 72                      # strips >= REP_END never use REP


def _is_rep(u):
    if u >= REP_END:
        return False
    return REP_FRAC == 1 or (REP_FRAC > 0 and (u % REP_FRAC) == 1)

# wc8 fp8 column map
C_CONV = 0                        # 3 taps x [2,96] = 576
C_FUSE = 576                      # 3 variants x [2,128] = 768
C_LBMS = 576 + 768                # 8 variants x [2,128] = 2048
W8TOT = C_LBMS + 2048
# wcb bf16 column map
B_LCW = 0                         # 8 variants x 96 = 768
B_SEL = 768                       # 96
B_W1 = 768 + 96                   # 96
B_W2 = 768 + 192                  # 128
B_PSBW = 768 + 320                # 2048 (partitions 96..125)
B_ONES = B_PSBW + 2048            # 512 cols of 1.0 at partition 48
WBTOT = B_ONES + 512
# fcol f32 columns: 0 fcb, 1 cvw, 2 cvb, 3 sigbias, 4..6 fuse bias variants
FCOLS = 7

_cache = {}


# ---------------------------------------------------------------------------
# host-side packing
# ---------------------------------------------------------------------------

def _conv_w8(fc_w):
    """[128, 576] fp8: tap d at cols 192d..192d+191, pair p block of 96:
    w[k=r*16+c, 192d+96p+i*16+oc] = fc_w[oc, 16p+c, r-i, d] for r-i in 0..2."""
    out = np.zeros((128, 576), np.float32)
    for d in range(3):
        for p in range(2):
            for i in range(SB):
                for ky in range(3):
                    r = i + ky
                    out[r * 16:r * 16 + 16,
                        192 * d + 96 * p + i * 16:192 * d + 96 * p + i * 16 + 16] = \
                        fc_w[:, 16 * p:16 * p + 16, ky, d].T
    return out.astype(F8)


def _fuse_w8(fuse_w):
    """[40, 768] fp8: 3 identical variants (edges handled by evac bias cols):
    w[k=r*5+yc, 256v+0*128+r*16+oc] = fuse_w[oc, yc]; pair1 block zero."""
    out = np.zeros((40, 768), np.float32)
    for v in range(3):
        for r in range(8):
            out[r * 5:r * 5 + 5, 256 * v + r * 16:256 * v + r * 16 + 16] = \
                fuse_w[:, :, 0, 0].T
    return out.astype(F8)


def _lbms_w8(fm_w):
    """[128, 2048] fp8 static part: variant g at cols 256g..: pair0
    [k=i*16+c, m=6g+i] = fm diff; rows 96..125 (bd) filled on device."""
    out = np.zeros((128, 2048), np.float32)
    d = fm_w[1, :, 0, 0] - fm_w[0, :, 0, 0]
    for g in range(8):
        for i in range(SB):
            out[i * 16:i * 16 + 16, 256 * g + 6 * g + i] = d
    return out.astype(F8)


def _wcb(cv_w, cv_b, se_w1, se_w2, bd_w):
    out = np.zeros((128, WBTOT), np.float32)
    # LCW: 8 variants [49 rows, 96]: rows 6g+i -> cv_w; row 48 (ones) -> cv_b
    for g in range(8):
        for i in range(SB):
            out[6 * g + i, B_LCW + 96 * g + i * 16:B_LCW + 96 * g + i * 16 + 16] = \
                cv_w[:, 0, 0, 0]
        for i in range(SB):
            out[48, B_LCW + 96 * g + i * 16:B_LCW + 96 * g + i * 16 + 16] = cv_b
    out[48, B_ONES:B_ONES + 512] = 1.0
    # SEL: [r*16+oc, oc] = 1/NPIX for r in 1..6
    for r in range(1, 7):
        for fc in range(16):
            out[r * 16 + fc, B_SEL + fc] = 1.0 / float(H * W)
    # W1
    out[0:16, B_W1:B_W1 + 16] = se_w1.T
    # W2: cols 96 + r*5 + yc <- se_w2.T (targets se_bc partitions 96..125)
    for r in range(SB):
        out[0:16, B_W2 + 96 + r * 5:B_W2 + 96 + r * 5 + 5] = se_w2.T
    # PSBW: partitions 96..125: [96 + r*5+yc, 256g + 64+6g+r] = bd diff
    dbd = bd_w[1, :, 0, 0] - bd_w[0, :, 0, 0]
    for g in range(8):
        for r in range(SB):
            out[96 + r * 5:96 + r * 5 + 5,
                B_PSBW + 256 * g + 64 + 6 * g + r] = dbd
    return out.astype(BF16)


def _fcol(fc_b, cv_w, cv_b, fm_b, bd_b, fuse_b):
    out = np.zeros((128, FCOLS), np.float32)
    for i in range(SB):
        out[i * 16:(i + 1) * 16, 0] = fc_b          # conv bias (+relu evac)
        out[i * 16:(i + 1) * 16, 1] = cv_w[:, 0, 0, 0]
        out[i * 16:(i + 1) * 16, 2] = cv_b
    for g in range(8):
        out[6 * g:6 * g + 6, 3] = fm_b[1] - fm_b[0]      # sigmoid bias: mask
        out[64 + 6 * g:64 + 6 * g + 6, 3] = bd_b[1] - bd_b[0]  # boundary
    for r in range(8):
        out[r * 16:r * 16 + 16, 4] = fuse_b          # interior fuse bias
        out[r * 16:r * 16 + 16, 5] = fuse_b if r > 0 else 0.0   # first tile
        out[r * 16:r * 16 + 16, 6] = fuse_b if r < 3 else 0.0   # last tile
    return out


def _pack_inputs(xb, yb):
    """xp [NT,128,514] fp8; yhp [NT,40,512] fp8; ycp [NT,30,512] fp8."""
    B = xb.shape[0]
    x8 = xb.astype(F8)
    y8 = yb.astype(F8)
    xpad = np.zeros((B, 16, SB * NT + 8, W + 2), F8)
    xpad[:, :, 1:H + 1, 1:W + 1] = x8
    ridx = SB * np.arange(NT)[:, None] + np.arange(8)[None, :]
    xp = xpad[:, :, ridx, :].transpose(0, 2, 3, 1, 4).reshape(B, NT, 128, XW)
    ypad = np.zeros((B, 5, SB * NT + 8, W), F8)
    ypad[:, :, 1:H + 1, :] = y8
    yhp = ypad[:, :, ridx, :].transpose(0, 2, 3, 1, 4).reshape(B, NT, 40, W)
    cidx = SB * np.arange(NT)[:, None] + 1 + np.arange(SB)[None, :]
    ycp = ypad[:, :, cidx, :].transpose(0, 2, 3, 1, 4).reshape(B, NT, 30, W)
    return (np.ascontiguousarray(xp), np.ascontiguousarray(yhp),
            np.ascontiguousarray(ycp))


# ---------------------------------------------------------------------------
# bass graph
# ---------------------------------------------------------------------------

def _pairs(v, pair_stride, n, base=0):
    """3-dim DR rhs AP [[part],[pair_stride,2],[1,n]] from a 2-dim view."""
    a = v.unsqueeze(1).copy()
    a.ap[1] = [pair_stride, 2]
    a.ap[2] = [1, n]
    return a


def _build():
    import concourse.bacc as bacc
    import concourse.tile as tile
    from concourse import mybir

    f32 = mybir.dt.float32
    bf16 = mybir.dt.bfloat16
    f8 = mybir.dt.float8e4
    AF = mybir.ActivationFunctionType
    ALU = mybir.AluOpType
    DR = mybir.MatmulPerfMode.DoubleRow

    def _pick(act_cost, dve_cost):
        bal = _cache.setdefault("_bal", [0.0, 0.0])
        if (bal[0] + act_cost) * ACT_W <= bal[1] + dve_cost:
            bal[0] += act_cost
            return 0
        bal[1] += dve_cost
        return 1

    nc = bacc.Bacc("TRN2", target_bir_lowering=False)
    xp_ext = nc.declare_dram_parameter("xp", [NT, 128, XW], f8, isOutput=False)
    yhp_ext = nc.declare_dram_parameter("yhp", [NT, 40, W], f8, isOutput=False)
    ycp_ext = nc.declare_dram_parameter("ycp", [NT, 30, W], f8, isOutput=False)
    wc8_ext = nc.declare_dram_parameter("wc8", [128, W8TOT], f8, isOutput=False)
    wcb_ext = nc.declare_dram_parameter("wcb", [128, WBTOT], bf16,
                                        isOutput=False)
    fcol_ext = nc.declare_dram_parameter("fcol", [128, FCOLS], f32,
                                         isOutput=False)
    out_ext = nc.declare_dram_parameter("out", [NT, 96, W], bf16,
                                        isOutput=True)

    with tile.TileContext(nc) as tc:
        with (
            tc.tile_pool(name="singles", bufs=1) as singles,
            tc.tile_pool(name="ps_fuse", bufs=2, space="PSUM") as ps_fuse,
            tc.tile_pool(name="ps_conv", bufs=2, space="PSUM") as ps_conv,
            tc.tile_pool(name="ps_lc", bufs=2, space="PSUM") as ps_lc,
        ):
            # ---------------- static tiles + warmup -----------------------
            wtile = singles.tile([128, 256], bf16, tag="wtile")
            nc.vector.memset(wtile[:, :], 0.0)
            wps = ps_conv.tile([96, 1024], f32, tag="conv", name="warmps")
            for i in range(N_WARM):
                nc.tensor.matmul(wps[0:96, 0:256], lhsT=wtile[:, 0:96],
                                 rhs=wtile[:, :], start=(i == 0),
                                 stop=(i == N_WARM - 1))

            wc8 = singles.tile([128, W8TOT], f8, tag="wc8")
            nc.sync.dma_start(out=wc8[:, :], in_=wc8_ext[:, :])
            wcb = singles.tile([128, WBTOT], bf16, tag="wcb")
            fcol = singles.tile([128, FCOLS], f32, tag="fcol")
            nc.sync.dma_start(out=fcol[:, :], in_=fcol_ext[:, :])

            def conv_lhsT(d):
                a = wc8[:, 192 * d:192 * (d + 1)].unsqueeze(1).copy()
                a.ap[1] = [96, 2]
                a.ap[2] = [1, 96]
                return a

            def fuse_lhsT(v):
                a = wc8[0:40, C_FUSE + 256 * v:C_FUSE + 256 * (v + 1)] \
                    .unsqueeze(1).copy()
                a.ap[1] = [128, 2]
                a.ap[2] = [1, 128]
                return a

            def lbms_lhsT(g):
                a = wc8[0:126, C_LBMS + 256 * g:C_LBMS + 256 * (g + 1)] \
                    .unsqueeze(1).copy()
                a.ap[1] = [128, 2]
                a.ap[2] = [1, 128]
                return a

            SELW = wcb[:, B_SEL:B_SEL + 96]
            W1L = wcb[:, B_W1:B_W1 + 96]
            W2R = wcb[:, B_W2:B_W2 + 128]

            # ---------------- data tiles ----------------------------------
            xf = singles.tile([128, NT * PW], f8, tag="xf")
            yh = singles.tile([40, RING_YH * XW], f8, tag="yh")
            fcc = singles.tile([128, NT * FW], f8, tag="fcc")
            og = [singles.tile([96, 8 * W], bf16, tag=f"og{k}", name=f"og{k}")
                  for k in (0, 1, 2)]
            sg = [singles.tile([112, W], bf16, tag=f"sg{k}", name=f"sg{k}")
                  for k in (0, 1)]
            sgB = [singles.tile([48, W], bf16, tag=f"sgB{k}", name=f"sgB{k}")
                   for k in (0, 1)]
            svg = [singles.tile([49, W], bf16, tag=f"svg{k}", name=f"svg{k}")
                   for k in (0, 1, 2)]
            rep = [singles.tile([96, W], bf16, tag=f"rep{k}", name=f"rep{k}")
                   for k in (0, 1, 2, 3)]
            Ra = singles.tile([128, NT], f32, tag="Ra")
            nc.vector.memset(Ra[:, :], 0.0)
            Rb = singles.tile([128, NT], f32, tag="Rb")
            nc.vector.memset(Rb[:, :], 0.0)

            # presets: XF F-block pad cols (t=0,513 per ring slot); yh pad
            # cols 512/513 per slot; fcc pad cols 512/513 per slot.
            m = xf[:, XW:XW + 1].unsqueeze(1).copy()
            m.ap[1] = [PW, NT]
            m.ap[2] = [XW - 1, 2]
            nc.gpsimd.memset(m, 0.0)
            m = yh[:, W:W + 1].unsqueeze(1).copy()
            m.ap[1] = [XW, RING_YH]
            m.ap[2] = [1, 2]
            nc.gpsimd.memset(m, 0.0)
            m = fcc[:, W:W + 1].unsqueeze(1).copy()
            m.ap[1] = [FW, NT]
            m.ap[2] = [1, 2]
            nc.gpsimd.memset(m, 0.0)

            # ---------------- phase A + fronts ----------------------------
            def xslot(t):
                return xf[:, t * PW:t * PW + PW]

            def issue_fuse(t):
                if t % 8 == 0:
                    n = min(8, NT - t)
                    nc.sync.dma_start(
                        out=xf[:, t * PW:(t + n) * PW].rearrange(
                            "p (s c) -> p s c", s=n)[:, :, 0:XW],
                        in_=xp_ext[t:t + n, :, :].rearrange("s p j -> p s j"))
                    s1 = (t % RING_YH)
                    nc.sync.dma_start(
                        out=yh[:, s1 * XW:(s1 + n) * XW].rearrange(
                            "p (s c) -> p s c", s=n)[:, :, 0:W],
                        in_=yhp_ext[t:t + n, :, :].rearrange("s p j -> p s j"))
                yv = yh[0:40, (t % RING_YH) * XW:(t % RING_YH) * XW + W]
                fps = ps_fuse.tile([128, 512], f32, tag="fuse")
                v = 1 if t == 0 else (2 if t == NT - 1 else 0)
                nc.tensor.matmul(fps[:, :], lhsT=fuse_lhsT(v),
                                 rhs=_pairs(yv, 2, W), start=True, stop=True,
                                 perf_mode=DR)
                fdst = xslot(t)[:, XW + 1:XW + 1 + W]
                bv = 4 if 0 < t < NT - 1 else (5 if t == 0 else 6)
                bias = fcol[:, bv:bv + 1]
                if _pick(612, 658) == 0:
                    nc.scalar.activation(out=fdst, in_=fps[:, :], func=AF.Relu,
                                         bias=bias, accum_out=Ra[:, t:t + 1])
                else:
                    nc.vector.tensor_scalar(out=fdst, in0=fps[:, :],
                                            scalar1=bias, scalar2=0.0,
                                            op0=ALU.add, op1=ALU.max,
                                            accum_out=Rb[:, t:t + 1])

            cpair = {}

            def issue_front(s):
                xv = xslot(s)
                CD = CONV_DEPTH
                if s % CD == 0:
                    cpair[0] = ps_conv.tile([96, CD * 512], f32, tag="conv",
                                            name="cps")
                cps = cpair[0]
                half = (s % CD) * 512
                for d in range(3):
                    nc.tensor.matmul(cps[:, half:half + 512],
                                     lhsT=conv_lhsT(d),
                                     rhs=_pairs(xv[:, d:], XW, W),
                                     start=(d == 0), stop=(d == 2),
                                     perf_mode=DR)
                nb = s % CD + 1
                if s % CD != CD - 1 and s != NT - 1:
                    return
                s0 = s - nb + 1
                dst = fcc[0:96, s0 * FW:s0 * FW + 1].unsqueeze(1).copy()
                dst.ap[1] = [FW, nb]
                dst.ap[2] = [1, W]
                src = cps[:, 0:nb * 512]
                if _pick(185 + nb * 427, 125 + nb * 533) == 0:
                    nc.scalar.activation(out=dst, in_=src, func=AF.Relu,
                                         bias=fcol[0:96, 0:1])
                else:
                    nc.vector.tensor_scalar(out=dst, in0=src,
                                            scalar1=fcol[0:96, 0:1],
                                            scalar2=0.0,
                                            op0=ALU.add, op1=ALU.max)

            for t in range(NT):
                if t == 2:
                    nc.sync.dma_start(out=wcb[:, :], in_=wcb_ext[:, :])
                    for k in (0, 1, 2):
                        nc.sync.dma_start(
                            out=svg[k][48:49, :],
                            in_=wcb[48:49, B_ONES:B_ONES + 512])
                issue_fuse(t)

            # ---------------- SE chain ------------------------------------
            R_bf = singles.tile([128, NT], bf16, tag="Rbf")
            nc.vector.tensor_add(out=R_bf[:, :], in0=Ra[:, :], in1=Rb[:, :])
            gps = ps_fuse.tile([96, NT], f32, tag="fuse")
            nc.tensor.matmul(gps[:, :], lhsT=SELW, rhs=R_bf[:, :],
                             start=True, stop=True)
            gap_f = singles.tile([96, 1], f32, tag="gapf")
            nc.vector.reduce_sum(out=gap_f[:, :], in_=gps[:, :],
                                 axis=mybir.AxisListType.X)
            gap_bf = singles.tile([128, 1], bf16, tag="gap")
            nc.vector.memset(gap_bf[:, :], 0.0)
            nc.vector.tensor_copy(out=gap_bf[0:96, :], in_=gap_f[:, :])
            hps = ps_fuse.tile([96, 1], f32, tag="fuse")
            nc.tensor.matmul(hps[:, :], lhsT=W1L, rhs=gap_bf[:, :],
                             start=True, stop=True)
            h_bf = singles.tile([128, 1], bf16, tag="hbf")
            nc.vector.memset(h_bf[:, :], 0.0)
            nc.scalar.activation(out=h_bf[0:96, :], in_=hps[:, :], func=AF.Relu)
            sps = ps_fuse.tile([128, 1], f32, tag="fuse")
            nc.tensor.matmul(sps[:, :], lhsT=W2R, rhs=h_bf[:, :],
                             start=True, stop=True)
            se_bc = singles.tile([128, 1], f32, tag="sebc")
            nc.scalar.activation(out=se_bc[:, :], in_=sps[:, :],
                                 func=AF.Sigmoid)
            # fill LBMS bd rows (96..125): wc8 <- PSBW * se
            nc.scalar.activation(out=wc8[96:126, C_LBMS:C_LBMS + 1024],
                                 in_=wcb[96:126, B_PSBW:B_PSBW + 1024],
                                 func=AF.Copy, scale=se_bc[96:126, :])
            nc.vector.tensor_scalar(
                out=wc8[96:126, C_LBMS + 1024:C_LBMS + 2048],
                in0=wcb[96:126, B_PSBW + 1024:B_PSBW + 2048],
                scalar1=se_bc[96:126, :], scalar2=0.0,
                op0=ALU.mult, op1=ALU.add)

            # ---------------- tails ---------------------------------------
            def issue_tail(u):
                g = u % 8
                G = u // 8
                gp = _cache.setdefault("_gp", {})
                if g == 0:
                    gp[G] = ps_fuse.tile([128, 512], f32, tag="fuse",
                                         name="grp")
                GP = gp[G]
                fv = fcc[0:126, u * FW:u * FW + W]
                nc.tensor.matmul(GP[:, :], lhsT=lbms_lhsT(g),
                                 rhs=_pairs(fv, 2, W), start=(g == 0),
                                 stop=(g == 7 or u == NT - 1), perf_mode=DR)
            def issue_chain(G):
                u = min(8 * G + 7, NT - 1)
                ng = u - 8 * G + 1           # strips in this group
                gp = _cache.setdefault("_gp", {})
                GP = gp[G]
                sgt, svt = sg[G % 2], svg[G % 3]
                np_ = 64 + 6 * ng
                bal = _cache.setdefault("_bal", [0.0, 0.0])
                bal[0] += 612.0
                nc.scalar.activation(out=sgt[0:np_, :], in_=GP[0:np_, :],
                                     func=AF.Sigmoid, bias=fcol[0:np_, 3:4])
                sbt = sgB[G % 2]
                ce = nc.gpsimd if CHAIN_POOL else nc.vector
                if not CHAIN_POOL:
                    bal[1] += 713.0
                ce.tensor_copy(out=sbt[0:6 * ng, :],
                               in_=sgt[64:64 + 6 * ng, :])
                ce.tensor_add(out=svt[0:6 * ng, :],
                              in0=sgt[0:6 * ng, :],
                              in1=sbt[0:6 * ng, :])
                ce.tensor_scalar(out=svt[0:6 * ng, :],
                                 in0=svt[0:6 * ng, :],
                                 scalar1=1.0, scalar2=0.0,
                                 op0=ALU.min, op1=ALU.add)

            def og_dma(G):
                ng = min(8, NT - 8 * G)
                nc.sync.dma_start(
                    out=out_ext[8 * G:8 * G + ng, :, :].rearrange(
                        "s p j -> p s j"),
                    in_=og[G % 3][0:96, 0:ng * W])

            def issue_final(uu):
                G = uu // 8
                gg = uu % 8
                svt = svg[G % 3]
                if True:
                    dst = og[G % 3][:, gg * W:gg * W + W]
                    if _is_rep(uu):
                        rt = rep[_cache.setdefault('_rr', [0])[0] % 4]
                        _cache['_rr'][0] += 1
                        src = svt[6 * gg:6 * gg + 6, :].unsqueeze(1).copy()
                        src.ap[1] = [0, 16]
                        nc.sync.dma_start(out=rt[:, :], in_=src)
                        k = _cache.setdefault("_repn", [0])
                        k[0] += 1
                        eng = nc.gpsimd if k[0] % 2 == 0 else nc.vector
                        eng.tensor_scalar(out=dst, in0=rt[:, :],
                                          scalar1=fcol[0:96, 1:2],
                                          scalar2=fcol[0:96, 2:3],
                                          op0=ALU.mult, op1=ALU.add)
                    else:
                        ops = ps_lc.tile([96, 512], f32, tag="lc")
                        nc.tensor.matmul(ops[:, :],
                                         lhsT=wcb[0:49, B_LCW + 96 * gg:
                                                  B_LCW + 96 * (gg + 1)],
                                         rhs=svt[0:49, :],
                                         start=True, stop=True)
                        if _pick(612, 658) == 0:
                            nc.scalar.activation(out=dst, in_=ops[:, :],
                                                 func=AF.Copy)
                        else:
                            nc.vector.tensor_copy(out=dst, in_=ops[:, :])

            for s0 in range(0, NT, 8):
                n = min(8, NT - s0)
                nc.sync.dma_start(
                    out=fcc[96:126, s0 * FW:(s0 + n) * FW].rearrange(
                        "p (s c) -> p s c", s=n)[:, :, 0:W],
                    in_=ycp_ext[s0:s0 + n, :, :].rearrange("s p j -> p s j"))
            done = set()
            for s in range(NT + TAIL_LAG + FIN_LAG + 16):
                if s < NT:
                    issue_front(s)
                u = s - TAIL_LAG
                if 0 <= u < NT:
                    issue_tail(u)
                uc = u - CHAIN_LAG
                if 0 <= uc < NT and (uc % 8 == 7 or uc == NT - 1) \
                        and ("c", uc // 8) not in done:
                    done.add(("c", uc // 8))
                    issue_chain(uc // 8)
                uf = u - CHAIN_LAG - FIN_LAG
                if 0 <= uf < NT:
                    issue_final(uf)
                    if uf % 8 == 7 or uf == NT - 1:
                        og_dma(uf // 8)
    nc.compile()
    for k in ("_gp", "_lcn", "_repn", "_bal", "_rr", "_rq"):
        _cache.pop(k, None)
    return nc


# ---------------------------------------------------------------------------
# entry point
# ---------------------------------------------------------------------------

LAST_RESULT = None


def prepare(x, y, fuse_w, fuse_b, se_w1, se_w2, bd_w, bd_b,
            fc_w, fc_b, fm_w, fm_b, cv_w, cv_b):
    if "nc" not in _cache:
        _cache["nc"] = _build()
    nc = _cache["nc"]
    g = {k: np.asarray(v, np.float32) for k, v in dict(
        fuse_w=fuse_w, fuse_b=fuse_b, se_w1=se_w1, se_w2=se_w2, bd_w=bd_w,
        bd_b=bd_b, fc_w=fc_w, fc_b=fc_b, fm_w=fm_w, fm_b=fm_b, cv_w=cv_w,
        cv_b=cv_b).items()}
    wc8 = np.zeros((128, W8TOT), F8)
    wc8[:, 0:576] = _conv_w8(g["fc_w"])
    wc8[0:40, C_FUSE:C_FUSE + 768] = _fuse_w8(g["fuse_w"])
    wc8[:, C_LBMS:C_LBMS + 2048] = _lbms_w8(g["fm_w"])
    wcb = _wcb(g["cv_w"], g["cv_b"], g["se_w1"], g["se_w2"], g["bd_w"])
    fcol = _fcol(g["fc_b"], g["cv_w"], g["cv_b"], g["fm_b"], g["bd_b"],
                 g["fuse_b"])
    xb = np.asarray(x, np.float32)
    yb = np.asarray(y, np.float32)
    xp, yhp, ycp = _pack_inputs(xb, yb)
    in_maps = [
        {"xp": xp[i], "yhp": yhp[i], "ycp": ycp[i],
         "wc8": wc8, "wcb": wcb, "fcol": fcol}
        for i in range(xb.shape[0])
    ]
    return nc, in_maps


def kernel(x, y, fuse_w, fuse_b, se_w1, se_w2, bd_w, bd_b,
           fc_w, fc_b, fm_w, fm_b, cv_w, cv_b):
    global LAST_RESULT
    from concourse.bass_utils import run_bass_kernel_spmd

    nc, in_maps = prepare(x, y, fuse_w, fuse_b, se_w1, se_w2, bd_w, bd_b,
                          fc_w, fc_b, fm_w, fm_b, cv_w, cv_b)
    res = run_bass_kernel_spmd(nc, in_maps, core_ids=list(range(8)))
    LAST_RESULT = res
    outs = []
    for i in range(len(in_maps)):
        ot = np.asarray(res.results[i]["out"], np.float32)   # [NT, 96, W]
        full = ot.reshape(NT, SB, 16, W).transpose(2, 0, 1, 3) \
                 .reshape(16, NT * SB, W)[:, :H, :]
        outs.append(full)
    return np.stack(outs)


# revision 6
# speedup vs baseline: 1.0175x; 1.0175x over previous
"""Trainium2 Bass kernel for nn_Boundary_Enchance (dense_cnn), v2.

Pure data parallel: core i processes batch image i.  Compute is fp8-e4m3
DoubleRow on the PE for the fuse 1x1, the 3x3 conv, and the merged
mask+boundary head (validated rel-err 0.004 on host); the final 1->16
expansion stays bf16 (fp8 weights there cost ~2% systematic error).

Layout tricks:
  - XF tiles [128, 1028] fp8: cols 0..513 = x (8 rows x 16ch, image cols
    -1..512, host-packed), cols 514..1027 = F = relu(fuse(y)) written by the
    fuse evacuation.  The 3x3 conv is THREE DoubleRow matmuls (one per dx
    tap): the DR pair (stride 514) contracts x and F simultaneously, K_eff
    = 256 = 8 rows x 32 ch.  dy taps ride the row-Toeplitz lhsT; dx taps
    are rhs base-column shifts.
  - fuse: one DR matmul per 8-row tile, pair (j, j+2) with zero second
    weights; bias folded into the evacuation tensor_scalar (per-tile bias
    column variants handle the image edges).
  - mask+boundary (LBM): per strip one DR matmul with M=128 accumulating 8
    strips into ONE psum bank (strip g -> mask logits at partitions 6g+i,
    boundary at 64+6g+i, zeros elsewhere).  Tail nonlinearities run ONCE
    per 8 strips: sigmoid (with per-partition bias = head biases), add,
    min -> sv group tile [48, 512].
  - final expansion per strip: either an LC bf16 matmul + psum evacuation,
    or a replicating SBUF DMA + per-partition affine (x cv_w + cv_b) on
    the DVE at 4x rate ("REP path") -- mixed to balance PE/DMA/engines.
"""

import numpy as np
import ml_dtypes

BF16 = ml_dtypes.bfloat16
F8 = ml_dtypes.float8_e4m3

H = 512
W = 512
SB = 6
NT = (H + SB - 1) // SB          # 86 strips
XW = 514                          # x / F block width (image cols -1..512)
PW = 2 * XW                       # XF tile width
FW = 514                          # fcc slot width (512 + 2 pad)
NG = (NT + 7) // 8                # strip groups of 8

RING_YH = 16
TAIL_LAG = 8                      # tails lag fronts in phase B
N_WARM = 20
CHAIN_POOL = False


REP_FRAC = 3                      # u % REP_FRAC == 1 -> REP; 0 disables
CONV_DEPTH = 2                    # strips per conv psum tile group
CHAIN_LAG = 3
FIN_LAG = 8
ACT_W = 1.0


REP_END = 72                      # strips >= REP_END never use REP


REP_MID = 0                       # strips in [REP_MID, REP_END) use u%2==1
OG_SPLIT = False


def _is_rep(u):
    if u >= REP_END:
        return False
    if REP_MID and u >= REP_MID:
        return (u % 2) == 1
    return REP_FRAC == 1 or (REP_FRAC > 0 and (u % REP_FRAC) == 1)

# wc8 fp8 column map
C_CONV = 0                        # 3 taps x [2,96] = 576
C_FUSE = 576                      # 3 variants x [2,128] = 768
C_LBMS = 576 + 768                # 8 variants x [2,128] = 2048
W8TOT = C_LBMS + 2048
# wcb bf16 column map
B_LCW = 0                         # 8 variants x 96 = 768
B_SEL = 768                       # 96
B_W1 = 768 + 96                   # 96
B_W2 = 768 + 192                  # 128
B_PSBW = 768 + 320                # 2048 (partitions 96..125)
B_ONES = B_PSBW + 2048            # 512 cols of 1.0 at partition 48
WBTOT = B_ONES + 512
# fcol f32 columns: 0 fcb, 1 cvw, 2 cvb, 3 sigbias, 4..6 fuse bias variants
FCOLS = 7

_cache = {}


# ---------------------------------------------------------------------------
# host-side packing
# ---------------------------------------------------------------------------

def _conv_w8(fc_w):
    """[128, 576] fp8: tap d at cols 192d..192d+191, pair p block of 96:
    w[k=r*16+c, 192d+96p+i*16+oc] = fc_w[oc, 16p+c, r-i, d] for r-i in 0..2."""
    out = np.zeros((128, 576), np.float32)
    for d in range(3):
        for p in range(2):
            for i in range(SB):
                for ky in range(3):
                    r = i + ky
                    out[r * 16:r * 16 + 16,
                        192 * d + 96 * p + i * 16:192 * d + 96 * p + i * 16 + 16] = \
                        fc_w[:, 16 * p:16 * p + 16, ky, d].T
    return out.astype(F8)


def _fuse_w8(fuse_w):
    """[40, 768] fp8: 3 identical variants (edges handled by evac bias cols):
    w[k=r*5+yc, 256v+0*128+r*16+oc] = fuse_w[oc, yc]; pair1 block zero."""
    out = np.zeros((40, 768), np.float32)
    for v in range(3):
        for r in range(8):
            out[r * 5:r * 5 + 5, 256 * v + r * 16:256 * v + r * 16 + 16] = \
                fuse_w[:, :, 0, 0].T
    return out.astype(F8)


def _lbms_w8(fm_w):
    """[128, 2048] fp8 static part: variant g at cols 256g..: pair0
    [k=i*16+c, m=6g+i] = fm diff; rows 96..125 (bd) filled on device."""
    out = np.zeros((128, 2048), np.float32)
    d = fm_w[1, :, 0, 0] - fm_w[0, :, 0, 0]
    for g in range(8):
        for i in range(SB):
            out[i * 16:i * 16 + 16, 256 * g + 6 * g + i] = d
    return out.astype(F8)


def _wcb(cv_w, cv_b, se_w1, se_w2, bd_w):
    out = np.zeros((128, WBTOT), np.float32)
    # LCW: 8 variants [49 rows, 96]: rows 6g+i -> cv_w; row 48 (ones) -> cv_b
    for g in range(8):
        for i in range(SB):
            out[6 * g + i, B_LCW + 96 * g + i * 16:B_LCW + 96 * g + i * 16 + 16] = \
                cv_w[:, 0, 0, 0]
        for i in range(SB):
            out[48, B_LCW + 96 * g + i * 16:B_LCW + 96 * g + i * 16 + 16] = cv_b
    out[48, B_ONES:B_ONES + 512] = 1.0
    # SEL: [r*16+oc, oc] = 1/NPIX for r in 1..6
    for r in range(1, 7):
        for fc in range(16):
            out[r * 16 + fc, B_SEL + fc] = 1.0 / float(H * W)
    # W1
    out[0:16, B_W1:B_W1 + 16] = se_w1.T
    # W2: cols 96 + r*5 + yc <- se_w2.T (targets se_bc partitions 96..125)
    for r in range(SB):
        out[0:16, B_W2 + 96 + r * 5:B_W2 + 96 + r * 5 + 5] = se_w2.T
    # PSBW: partitions 96..125: [96 + r*5+yc, 256g + 64+6g+r] = bd diff
    dbd = bd_w[1, :, 0, 0] - bd_w[0, :, 0, 0]
    for g in range(8):
        for r in range(SB):
            out[96 + r * 5:96 + r * 5 + 5,
                B_PSBW + 256 * g + 64 + 6 * g + r] = dbd
    return out.astype(BF16)


def _fcol(fc_b, cv_w, cv_b, fm_b, bd_b, fuse_b):
    out = np.zeros((128, FCOLS), np.float32)
    for i in range(SB):
        out[i * 16:(i + 1) * 16, 0] = fc_b          # conv bias (+relu evac)
        out[i * 16:(i + 1) * 16, 1] = cv_w[:, 0, 0, 0]
        out[i * 16:(i + 1) * 16, 2] = cv_b
    for g in range(8):
        out[6 * g:6 * g + 6, 3] = fm_b[1] - fm_b[0]      # sigmoid bias: mask
        out[64 + 6 * g:64 + 6 * g + 6, 3] = bd_b[1] - bd_b[0]  # boundary
    for r in range(8):
        out[r * 16:r * 16 + 16, 4] = fuse_b          # interior fuse bias
        out[r * 16:r * 16 + 16, 5] = fuse_b if r > 0 else 0.0   # first tile
        out[r * 16:r * 16 + 16, 6] = fuse_b if r < 3 else 0.0   # last tile
    return out


def _pack_inputs(xb, yb):
    """xp [NT,128,514] fp8; yhp [NT,40,512] fp8; ycp [NT,30,512] fp8."""
    B = xb.shape[0]
    x8 = xb.astype(F8)
    y8 = yb.astype(F8)
    xpad = np.zeros((B, 16, SB * NT + 8, W + 2), F8)
    xpad[:, :, 1:H + 1, 1:W + 1] = x8
    ridx = SB * np.arange(NT)[:, None] + np.arange(8)[None, :]
    xp = xpad[:, :, ridx, :].transpose(0, 2, 3, 1, 4).reshape(B, NT, 128, XW)
    ypad = np.zeros((B, 5, SB * NT + 8, W), F8)
    ypad[:, :, 1:H + 1, :] = y8
    yhp = ypad[:, :, ridx, :].transpose(0, 2, 3, 1, 4).reshape(B, NT, 40, W)
    cidx = SB * np.arange(NT)[:, None] + 1 + np.arange(SB)[None, :]
    ycp = ypad[:, :, cidx, :].transpose(0, 2, 3, 1, 4).reshape(B, NT, 30, W)
    return (np.ascontiguousarray(xp), np.ascontiguousarray(yhp),
            np.ascontiguousarray(ycp))


# ---------------------------------------------------------------------------
# bass graph
# ---------------------------------------------------------------------------

def _pairs(v, pair_stride, n, base=0):
    """3-dim DR rhs AP [[part],[pair_stride,2],[1,n]] from a 2-dim view."""
    a = v.unsqueeze(1).copy()
    a.ap[1] = [pair_stride, 2]
    a.ap[2] = [1, n]
    return a


def _build():
    import concourse.bacc as bacc
    import concourse.tile as tile
    from concourse import mybir

    f32 = mybir.dt.float32
    bf16 = mybir.dt.bfloat16
    f8 = mybir.dt.float8e4
    AF = mybir.ActivationFunctionType
    ALU = mybir.AluOpType
    DR = mybir.MatmulPerfMode.DoubleRow

    def _pick(act_cost, dve_cost):
        bal = _cache.setdefault("_bal", [0.0, 0.0])
        if (bal[0] + act_cost) * ACT_W <= bal[1] + dve_cost:
            bal[0] += act_cost
            return 0
        bal[1] += dve_cost
        return 1

    nc = bacc.Bacc("TRN2", target_bir_lowering=False)
    xp_ext = nc.declare_dram_parameter("xp", [NT, 128, XW], f8, isOutput=False)
    yhp_ext = nc.declare_dram_parameter("yhp", [NT, 40, W], f8, isOutput=False)
    ycp_ext = nc.declare_dram_parameter("ycp", [NT, 30, W], f8, isOutput=False)
    wc8_ext = nc.declare_dram_parameter("wc8", [128, W8TOT], f8, isOutput=False)
    wcb_ext = nc.declare_dram_parameter("wcb", [128, WBTOT], bf16,
                                        isOutput=False)
    fcol_ext = nc.declare_dram_parameter("fcol", [128, FCOLS], f32,
                                         isOutput=False)
    out_ext = nc.declare_dram_parameter("out", [NT, 96, W], bf16,
                                        isOutput=True)

    with tile.TileContext(nc) as tc:
        with (
            tc.tile_pool(name="singles", bufs=1) as singles,
            tc.tile_pool(name="ps_fuse", bufs=2, space="PSUM") as ps_fuse,
            tc.tile_pool(name="ps_conv", bufs=2, space="PSUM") as ps_conv,
            tc.tile_pool(name="ps_lc", bufs=2, space="PSUM") as ps_lc,
        ):
            # ---------------- static tiles + warmup -----------------------
            wtile = singles.tile([128, 256], bf16, tag="wtile")
            nc.vector.memset(wtile[:, :], 0.0)
            wps = ps_conv.tile([96, 1024], f32, tag="conv", name="warmps")
            for i in range(N_WARM):
                nc.tensor.matmul(wps[0:96, 0:256], lhsT=wtile[:, 0:96],
                                 rhs=wtile[:, :], start=(i == 0),
                                 stop=(i == N_WARM - 1))

            wc8 = singles.tile([128, W8TOT], f8, tag="wc8")
            nc.sync.dma_start(out=wc8[:, :], in_=wc8_ext[:, :])
            wcb = singles.tile([128, WBTOT], bf16, tag="wcb")
            fcol = singles.tile([128, FCOLS], f32, tag="fcol")
            nc.sync.dma_start(out=fcol[:, :], in_=fcol_ext[:, :])

            def conv_lhsT(d):
                a = wc8[:, 192 * d:192 * (d + 1)].unsqueeze(1).copy()
                a.ap[1] = [96, 2]
                a.ap[2] = [1, 96]
                return a

            def fuse_lhsT(v):
                a = wc8[0:40, C_FUSE + 256 * v:C_FUSE + 256 * (v + 1)] \
                    .unsqueeze(1).copy()
                a.ap[1] = [128, 2]
                a.ap[2] = [1, 128]
                return a

            def lbms_lhsT(g):
                a = wc8[0:126, C_LBMS + 256 * g:C_LBMS + 256 * (g + 1)] \
                    .unsqueeze(1).copy()
                a.ap[1] = [128, 2]
                a.ap[2] = [1, 128]
                return a

            SELW = wcb[:, B_SEL:B_SEL + 96]
            W1L = wcb[:, B_W1:B_W1 + 96]
            W2R = wcb[:, B_W2:B_W2 + 128]

            # ---------------- data tiles ----------------------------------
            xf = singles.tile([128, NT * PW], f8, tag="xf")
            yh = singles.tile([40, RING_YH * XW], f8, tag="yh")
            fcc = singles.tile([128, NT * FW], f8, tag="fcc")
            og = [singles.tile([96, 8 * W], bf16, tag=f"og{k}", name=f"og{k}")
                  for k in (0, 1, 2)]
            sg = [singles.tile([112, W], bf16, tag=f"sg{k}", name=f"sg{k}")
                  for k in (0, 1)]
            sgB = [singles.tile([48, W], bf16, tag=f"sgB{k}", name=f"sgB{k}")
                   for k in (0, 1)]
            svg = [singles.tile([49, W], bf16, tag=f"svg{k}", name=f"svg{k}")
                   for k in (0, 1, 2)]
            rep = [singles.tile([96, W], bf16, tag=f"rep{k}", name=f"rep{k}")
                   for k in (0, 1, 2, 3)]
            Ra = singles.tile([128, NT], f32, tag="Ra")
            nc.vector.memset(Ra[:, :], 0.0)
            Rb = singles.tile([128, NT], f32, tag="Rb")
            nc.vector.memset(Rb[:, :], 0.0)

            # presets: XF F-block pad cols (t=0,513 per ring slot); yh pad
            # cols 512/513 per slot; fcc pad cols 512/513 per slot.
            m = xf[:, XW:XW + 1].unsqueeze(1).copy()
            m.ap[1] = [PW, NT]
            m.ap[2] = [XW - 1, 2]
            nc.gpsimd.memset(m, 0.0)
            m = yh[:, W:W + 1].unsqueeze(1).copy()
            m.ap[1] = [XW, RING_YH]
            m.ap[2] = [1, 2]
            nc.gpsimd.memset(m, 0.0)
            m = fcc[:, W:W + 1].unsqueeze(1).copy()
            m.ap[1] = [FW, NT]
            m.ap[2] = [1, 2]
            nc.gpsimd.memset(m, 0.0)

            # ---------------- phase A + fronts ----------------------------
            def xslot(t):
                return xf[:, t * PW:t * PW + PW]

            def issue_fuse(t):
                if t % 8 == 0:
                    n = min(8, NT - t)
                    nc.sync.dma_start(
                        out=xf[:, t * PW:(t + n) * PW].rearrange(
                            "p (s c) -> p s c", s=n)[:, :, 0:XW],
                        in_=xp_ext[t:t + n, :, :].rearrange("s p j -> p s j"))
                    s1 = (t % RING_YH)
                    nc.sync.dma_start(
                        out=yh[:, s1 * XW:(s1 + n) * XW].rearrange(
                            "p (s c) -> p s c", s=n)[:, :, 0:W],
                        in_=yhp_ext[t:t + n, :, :].rearrange("s p j -> p s j"))
                yv = yh[0:40, (t % RING_YH) * XW:(t % RING_YH) * XW + W]
                fps = ps_fuse.tile([128, 512], f32, tag="fuse")
                v = 1 if t == 0 else (2 if t == NT - 1 else 0)
                nc.tensor.matmul(fps[:, :], lhsT=fuse_lhsT(v),
                                 rhs=_pairs(yv, 2, W), start=True, stop=True,
                                 perf_mode=DR)
                fdst = xslot(t)[:, XW + 1:XW + 1 + W]
                bv = 4 if 0 < t < NT - 1 else (5 if t == 0 else 6)
                bias = fcol[:, bv:bv + 1]
                if _pick(612, 658) == 0:
                    nc.scalar.activation(out=fdst, in_=fps[:, :], func=AF.Relu,
                                         bias=bias, accum_out=Ra[:, t:t + 1])
                else:
                    nc.vector.tensor_scalar(out=fdst, in0=fps[:, :],
                                            scalar1=bias, scalar2=0.0,
                                            op0=ALU.add, op1=ALU.max,
                                            accum_out=Rb[:, t:t + 1])

            cpair = {}

            def issue_front(s):
                xv = xslot(s)
                CD = CONV_DEPTH
                if s % CD == 0:
                    cpair[0] = ps_conv.tile([96, CD * 512], f32, tag="conv",
                                            name="cps")
                cps = cpair[0]
                half = (s % CD) * 512
                for d in range(3):
                    nc.tensor.matmul(cps[:, half:half + 512],
                                     lhsT=conv_lhsT(d),
                                     rhs=_pairs(xv[:, d:], XW, W),
                                     start=(d == 0), stop=(d == 2),
                                     perf_mode=DR)
                nb = s % CD + 1
                if s % CD != CD - 1 and s != NT - 1:
                    return
                s0 = s - nb + 1
                dst = fcc[0:96, s0 * FW:s0 * FW + 1].unsqueeze(1).copy()
                dst.ap[1] = [FW, nb]
                dst.ap[2] = [1, W]
                src = cps[:, 0:nb * 512]
                if _pick(185 + nb * 427, 125 + nb * 533) == 0:
                    nc.scalar.activation(out=dst, in_=src, func=AF.Relu,
                                         bias=fcol[0:96, 0:1])
                else:
                    nc.vector.tensor_scalar(out=dst, in0=src,
                                            scalar1=fcol[0:96, 0:1],
                                            scalar2=0.0,
                                            op0=ALU.add, op1=ALU.max)

            for t in range(NT):
                if t == 2:
                    nc.sync.dma_start(out=wcb[:, :], in_=wcb_ext[:, :])
                    for k in (0, 1, 2):
                        nc.sync.dma_start(
                            out=svg[k][48:49, :],
                            in_=wcb[48:49, B_ONES:B_ONES + 512])
                issue_fuse(t)

            # ---------------- SE chain ------------------------------------
            R_bf = singles.tile([128, NT], bf16, tag="Rbf")
            nc.vector.tensor_add(out=R_bf[:, :], in0=Ra[:, :], in1=Rb[:, :])
            gps = ps_fuse.tile([96, NT], f32, tag="fuse")
            nc.tensor.matmul(gps[:, :], lhsT=SELW, rhs=R_bf[:, :],
                             start=True, stop=True)
            gap_f = singles.tile([96, 1], f32, tag="gapf")
            nc.vector.reduce_sum(out=gap_f[:, :], in_=gps[:, :],
                                 axis=mybir.AxisListType.X)
            gap_bf = singles.tile([128, 1], bf16, tag="gap")
            nc.vector.memset(gap_bf[:, :], 0.0)
            nc.vector.tensor_copy(out=gap_bf[0:96, :], in_=gap_f[:, :])
            hps = ps_fuse.tile([96, 1], f32, tag="fuse")
            nc.tensor.matmul(hps[:, :], lhsT=W1L, rhs=gap_bf[:, :],
                             start=True, stop=True)
            h_bf = singles.tile([128, 1], bf16, tag="hbf")
            nc.vector.memset(h_bf[:, :], 0.0)
            nc.scalar.activation(out=h_bf[0:96, :], in_=hps[:, :], func=AF.Relu)
            sps = ps_fuse.tile([128, 1], f32, tag="fuse")
            nc.tensor.matmul(sps[:, :], lhsT=W2R, rhs=h_bf[:, :],
                             start=True, stop=True)
            se_bc = singles.tile([128, 1], f32, tag="sebc")
            nc.scalar.activation(out=se_bc[:, :], in_=sps[:, :],
                                 func=AF.Sigmoid)
            # fill LBMS bd rows (96..125): wc8 <- PSBW * se
            nc.scalar.activation(out=wc8[96:126, C_LBMS:C_LBMS + 1024],
                                 in_=wcb[96:126, B_PSBW:B_PSBW + 1024],
                                 func=AF.Copy, scale=se_bc[96:126, :])
            nc.vector.tensor_scalar(
                out=wc8[96:126, C_LBMS + 1024:C_LBMS + 2048],
                in0=wcb[96:126, B_PSBW + 1024:B_PSBW + 2048],
                scalar1=se_bc[96:126, :], scalar2=0.0,
                op0=ALU.mult, op1=ALU.add)

            # ---------------- tails ---------------------------------------
            def issue_tail(u):
                g = u % 8
                G = u // 8
                gp = _cache.setdefault("_gp", {})
                if g == 0:
                    gp[G] = ps_fuse.tile([128, 512], f32, tag="fuse",
                                         name="grp")
                GP = gp[G]
                fv = fcc[0:126, u * FW:u * FW + W]
                nc.tensor.matmul(GP[:, :], lhsT=lbms_lhsT(g),
                                 rhs=_pairs(fv, 2, W), start=(g == 0),
                                 stop=(g == 7 or u == NT - 1), perf_mode=DR)
            def issue_chain(G):
                u = min(8 * G + 7, NT - 1)
                ng = u - 8 * G + 1           # strips in this group
                gp = _cache.setdefault("_gp", {})
                GP = gp[G]
                sgt, svt = sg[G % 2], svg[G % 3]
                np_ = 64 + 6 * ng
                bal = _cache.setdefault("_bal", [0.0, 0.0])
                bal[0] += 612.0
                nc.scalar.activation(out=sgt[0:np_, :], in_=GP[0:np_, :],
                                     func=AF.Sigmoid, bias=fcol[0:np_, 3:4])
                sbt = sgB[G % 2]
                ce = nc.gpsimd if CHAIN_POOL else nc.vector
                if not CHAIN_POOL:
                    bal[1] += 713.0
                ce.tensor_copy(out=sbt[0:6 * ng, :],
                               in_=sgt[64:64 + 6 * ng, :])
                ce.tensor_add(out=svt[0:6 * ng, :],
                              in0=sgt[0:6 * ng, :],
                              in1=sbt[0:6 * ng, :])
                ce.tensor_scalar(out=svt[0:6 * ng, :],
                                 in0=svt[0:6 * ng, :],
                                 scalar1=1.0, scalar2=0.0,
                                 op0=ALU.min, op1=ALU.add)

            def og_dma(G, half=None):
                ng = min(8, NT - 8 * G)
                lo, hi = 0, ng
                if half == 0:
                    hi = min(4, ng)
                elif half == 1:
                    lo = 4
                    if ng <= 4:
                        return
                nc.sync.dma_start(
                    out=out_ext[8 * G + lo:8 * G + hi, :, :].rearrange(
                        "s p j -> p s j"),
                    in_=og[G % 3][0:96, lo * W:hi * W])

            def issue_final(uu):
                G = uu // 8
                gg = uu % 8
                svt = svg[G % 3]
                if True:
                    dst = og[G % 3][:, gg * W:gg * W + W]
                    if _is_rep(uu):
                        rt = rep[_cache.setdefault('_rr', [0])[0] % 4]
                        _cache['_rr'][0] += 1
                        src = svt[6 * gg:6 * gg + 6, :].unsqueeze(1).copy()
                        src.ap[1] = [0, 16]
                        nc.sync.dma_start(out=rt[:, :], in_=src)
                        k = _cache.setdefault("_repn", [0])
                        k[0] += 1
                        eng = nc.gpsimd if k[0] % 2 == 0 else nc.vector
                        eng.tensor_scalar(out=dst, in0=rt[:, :],
                                          scalar1=fcol[0:96, 1:2],
                                          scalar2=fcol[0:96, 2:3],
                                          op0=ALU.mult, op1=ALU.add)
                    else:
                        ops = ps_lc.tile([96, 512], f32, tag="lc")
                        nc.tensor.matmul(ops[:, :],
                                         lhsT=wcb[0:49, B_LCW + 96 * gg:
                                                  B_LCW + 96 * (gg + 1)],
                                         rhs=svt[0:49, :],
                                         start=True, stop=True)
                        if _pick(612, 658) == 0:
                            nc.scalar.activation(out=dst, in_=ops[:, :],
                                                 func=AF.Copy)
                        else:
                            nc.vector.tensor_copy(out=dst, in_=ops[:, :])

            for s0 in range(0, NT, 8):
                n = min(8, NT - s0)
                nc.sync.dma_start(
                    out=fcc[96:126, s0 * FW:(s0 + n) * FW].rearrange(
                        "p (s c) -> p s c", s=n)[:, :, 0:W],
                    in_=ycp_ext[s0:s0 + n, :, :].rearrange("s p j -> p s j"))
            tc = [0]
            fc_ = [0]
            cc = [0]
            vu = [-1]
            s = 0
            while fc_[0] < NT:
                if s < NT:
                    issue_front(s)
                pace = 1 if s < NT else 2
                if s >= TAIL_LAG:
                    vu[0] += pace
                for _ in range(pace):
                    if tc[0] < NT and tc[0] <= vu[0]:
                        issue_tail(tc[0])
                        tc[0] += 1
                while cc[0] < NG and vu[0] - min(8 * cc[0] + 7, NT - 1) \
                        >= CHAIN_LAG:
                    issue_chain(cc[0])
                    cc[0] += 1
                uf_max = vu[0] - CHAIN_LAG - FIN_LAG
                nfin = pace
                while fc_[0] <= uf_max and fc_[0] < NT and nfin > 0 \
                        and fc_[0] // 8 < cc[0]:
                    uf = fc_[0]
                    fc_[0] += 1
                    nfin -= 1
                    issue_final(uf)
                    if uf % 8 == 7 or uf == NT - 1:
                        og_dma(uf // 8)
                s += 1
    nc.compile()
    for k in ("_gp", "_lcn", "_repn", "_bal", "_rr", "_rq", "_fc", "_cc"):
        _cache.pop(k, None)
    return nc


# ---------------------------------------------------------------------------
# entry point
# ---------------------------------------------------------------------------

LAST_RESULT = None


def prepare(x, y, fuse_w, fuse_b, se_w1, se_w2, bd_w, bd_b,
            fc_w, fc_b, fm_w, fm_b, cv_w, cv_b):
    if "nc" not in _cache:
        _cache["nc"] = _build()
    nc = _cache["nc"]
    g = {k: np.asarray(v, np.float32) for k, v in dict(
        fuse_w=fuse_w, fuse_b=fuse_b, se_w1=se_w1, se_w2=se_w2, bd_w=bd_w,
        bd_b=bd_b, fc_w=fc_w, fc_b=fc_b, fm_w=fm_w, fm_b=fm_b, cv_w=cv_w,
        cv_b=cv_b).items()}
    wc8 = np.zeros((128, W8TOT), F8)
    wc8[:, 0:576] = _conv_w8(g["fc_w"])
    wc8[0:40, C_FUSE:C_FUSE + 768] = _fuse_w8(g["fuse_w"])
    wc8[:, C_LBMS:C_LBMS + 2048] = _lbms_w8(g["fm_w"])
    wcb = _wcb(g["cv_w"], g["cv_b"], g["se_w1"], g["se_w2"], g["bd_w"])
    fcol = _fcol(g["fc_b"], g["cv_w"], g["cv_b"], g["fm_b"], g["bd_b"],
                 g["fuse_b"])
    xb = np.asarray(x, np.float32)
    yb = np.asarray(y, np.float32)
    xp, yhp, ycp = _pack_inputs(xb, yb)
    in_maps = [
        {"xp": xp[i], "yhp": yhp[i], "ycp": ycp[i],
         "wc8": wc8, "wcb": wcb, "fcol": fcol}
        for i in range(xb.shape[0])
    ]
    return nc, in_maps


def kernel(x, y, fuse_w, fuse_b, se_w1, se_w2, bd_w, bd_b,
           fc_w, fc_b, fm_w, fm_b, cv_w, cv_b):
    global LAST_RESULT
    from concourse.bass_utils import run_bass_kernel_spmd

    nc, in_maps = prepare(x, y, fuse_w, fuse_b, se_w1, se_w2, bd_w, bd_b,
                          fc_w, fc_b, fm_w, fm_b, cv_w, cv_b)
    res = run_bass_kernel_spmd(nc, in_maps, core_ids=list(range(8)))
    LAST_RESULT = res
    outs = []
    for i in range(len(in_maps)):
        ot = np.asarray(res.results[i]["out"], np.float32)   # [NT, 96, W]
        full = ot.reshape(NT, SB, 16, W).transpose(2, 0, 1, 3) \
                 .reshape(16, NT * SB, W)[:, :H, :]
        outs.append(full)
    return np.stack(outs)


# revision 7
# speedup vs baseline: 1.0360x; 1.0182x over previous
"""Trainium2 Bass kernel for nn_Boundary_Enchance (dense_cnn), v2.

Pure data parallel: core i processes batch image i.  Compute is fp8-e4m3
DoubleRow on the PE for the fuse 1x1, the 3x3 conv, and the merged
mask+boundary head (validated rel-err 0.004 on host); the final 1->16
expansion stays bf16 (fp8 weights there cost ~2% systematic error).

Layout tricks:
  - XF tiles [128, 1028] fp8: cols 0..513 = x (8 rows x 16ch, image cols
    -1..512, host-packed), cols 514..1027 = F = relu(fuse(y)) written by the
    fuse evacuation.  The 3x3 conv is THREE DoubleRow matmuls (one per dx
    tap): the DR pair (stride 514) contracts x and F simultaneously, K_eff
    = 256 = 8 rows x 32 ch.  dy taps ride the row-Toeplitz lhsT; dx taps
    are rhs base-column shifts.
  - fuse: one DR matmul per 8-row tile, pair (j, j+2) with zero second
    weights; bias folded into the evacuation tensor_scalar (per-tile bias
    column variants handle the image edges).
  - mask+boundary (LBM): per strip one DR matmul with M=128 accumulating 8
    strips into ONE psum bank (strip g -> mask logits at partitions 6g+i,
    boundary at 64+6g+i, zeros elsewhere).  Tail nonlinearities run ONCE
    per 8 strips: sigmoid (with per-partition bias = head biases), add,
    min -> sv group tile [48, 512].
  - final expansion per strip: either an LC bf16 matmul + psum evacuation,
    or a replicating SBUF DMA + per-partition affine (x cv_w + cv_b) on
    the DVE at 4x rate ("REP path") -- mixed to balance PE/DMA/engines.
"""

import numpy as np
import ml_dtypes

BF16 = ml_dtypes.bfloat16
F8 = ml_dtypes.float8_e4m3

H = 512
W = 512
SB = 6
NT = (H + SB - 1) // SB          # 86 strips
XW = 514                          # x / F block width (image cols -1..512)
PW = 2 * XW                       # XF tile width
FW = 514                          # fcc slot width (512 + 2 pad)
NG = (NT + 7) // 8                # strip groups of 8

RING_YH = 16
TAIL_LAG = 8                      # tails lag fronts in phase B
N_WARM = 20
CHAIN_POOL = False


REP_FRAC = 3                      # u % REP_FRAC == 1 -> REP; 0 disables
CONV_DEPTH = 2                    # strips per conv psum tile group
CHAIN_LAG = 3
FIN_LAG = 8
ACT_W = 0.97


REP_END = 72                      # strips >= REP_END never use REP


REP_MID = 0                       # strips in [REP_MID, REP_END) use u%2==1
OG_SPLIT = False


def _is_rep(u):
    if u >= REP_END:
        return False
    if REP_MID and u >= REP_MID:
        return (u % 2) == 1
    return REP_FRAC == 1 or (REP_FRAC > 0 and (u % REP_FRAC) == 1)

# wc8 fp8 column map
C_CONV = 0                        # 3 taps x [2,96] = 576
C_FUSE = 576                      # 3 variants x [2,128] = 768
C_LBMS = 576 + 768                # 8 variants x [2,128] = 2048
W8TOT = C_LBMS + 2048
# wcb bf16 column map
B_LCW = 0                         # 8 variants x 96 = 768
B_SEL = 768                       # 96
B_W1 = 768 + 96                   # 96
B_W2 = 768 + 192                  # 128
B_PSBW = 768 + 320                # 2048 (partitions 96..125)
B_ONES = B_PSBW + 2048            # 512 cols of 1.0 at partition 48
WBTOT = B_ONES + 512
# fcol f32 columns: 0 fcb, 1 cvw, 2 cvb, 3 sigbias, 4..6 fuse bias variants
FCOLS = 7

_cache = {}


# ---------------------------------------------------------------------------
# host-side packing
# ---------------------------------------------------------------------------

def _conv_w8(fc_w):
    """[128, 576] fp8: tap d at cols 192d..192d+191, pair p block of 96:
    w[k=r*16+c, 192d+96p+i*16+oc] = fc_w[oc, 16p+c, r-i, d] for r-i in 0..2."""
    out = np.zeros((128, 576), np.float32)
    for d in range(3):
        for p in range(2):
            for i in range(SB):
                for ky in range(3):
                    r = i + ky
                    out[r * 16:r * 16 + 16,
                        192 * d + 96 * p + i * 16:192 * d + 96 * p + i * 16 + 16] = \
                        fc_w[:, 16 * p:16 * p + 16, ky, d].T
    return out.astype(F8)


def _fuse_w8(fuse_w):
    """[40, 768] fp8: 3 identical variants (edges handled by evac bias cols):
    w[k=r*5+yc, 256v+0*128+r*16+oc] = fuse_w[oc, yc]; pair1 block zero."""
    out = np.zeros((40, 768), np.float32)
    for v in range(3):
        for r in range(8):
            out[r * 5:r * 5 + 5, 256 * v + r * 16:256 * v + r * 16 + 16] = \
                fuse_w[:, :, 0, 0].T
    return out.astype(F8)


def _lbms_w8(fm_w):
    """[128, 2048] fp8 static part: variant g at cols 256g..: pair0
    [k=i*16+c, m=6g+i] = fm diff; rows 96..125 (bd) filled on device."""
    out = np.zeros((128, 2048), np.float32)
    d = fm_w[1, :, 0, 0] - fm_w[0, :, 0, 0]
    for g in range(8):
        for i in range(SB):
            out[i * 16:i * 16 + 16, 256 * g + 6 * g + i] = d
    return out.astype(F8)


def _wcb(cv_w, cv_b, se_w1, se_w2, bd_w):
    out = np.zeros((128, WBTOT), np.float32)
    # LCW: 8 variants [49 rows, 96]: rows 6g+i -> cv_w; row 48 (ones) -> cv_b
    for g in range(8):
        for i in range(SB):
            out[6 * g + i, B_LCW + 96 * g + i * 16:B_LCW + 96 * g + i * 16 + 16] = \
                cv_w[:, 0, 0, 0]
        for i in range(SB):
            out[48, B_LCW + 96 * g + i * 16:B_LCW + 96 * g + i * 16 + 16] = cv_b
    out[48, B_ONES:B_ONES + 512] = 1.0
    # SEL: [r*16+oc, oc] = 1/NPIX for r in 1..6
    for r in range(1, 7):
        for fc in range(16):
            out[r * 16 + fc, B_SEL + fc] = 1.0 / float(H * W)
    # W1
    out[0:16, B_W1:B_W1 + 16] = se_w1.T
    # W2: cols 96 + r*5 + yc <- se_w2.T (targets se_bc partitions 96..125)
    for r in range(SB):
        out[0:16, B_W2 + 96 + r * 5:B_W2 + 96 + r * 5 + 5] = se_w2.T
    # PSBW: partitions 96..125: [96 + r*5+yc, 256g + 64+6g+r] = bd diff
    dbd = bd_w[1, :, 0, 0] - bd_w[0, :, 0, 0]
    for g in range(8):
        for r in range(SB):
            out[96 + r * 5:96 + r * 5 + 5,
                B_PSBW + 256 * g + 64 + 6 * g + r] = dbd
    return out.astype(BF16)


def _fcol(fc_b, cv_w, cv_b, fm_b, bd_b, fuse_b):
    out = np.zeros((128, FCOLS), np.float32)
    for i in range(SB):
        out[i * 16:(i + 1) * 16, 0] = fc_b          # conv bias (+relu evac)
        out[i * 16:(i + 1) * 16, 1] = cv_w[:, 0, 0, 0]
        out[i * 16:(i + 1) * 16, 2] = cv_b
    for g in range(8):
        out[6 * g:6 * g + 6, 3] = fm_b[1] - fm_b[0]      # sigmoid bias: mask
        out[64 + 6 * g:64 + 6 * g + 6, 3] = bd_b[1] - bd_b[0]  # boundary
    for r in range(8):
        out[r * 16:r * 16 + 16, 4] = fuse_b          # interior fuse bias
        out[r * 16:r * 16 + 16, 5] = fuse_b if r > 0 else 0.0   # first tile
        out[r * 16:r * 16 + 16, 6] = fuse_b if r < 3 else 0.0   # last tile
    return out


def _pack_inputs(xb, yb):
    """xp [NT,128,514] fp8; yhp [NT,40,512] fp8; ycp [NT,30,512] fp8."""
    B = xb.shape[0]
    x8 = xb.astype(F8)
    y8 = yb.astype(F8)
    xpad = np.zeros((B, 16, SB * NT + 8, W + 2), F8)
    xpad[:, :, 1:H + 1, 1:W + 1] = x8
    ridx = SB * np.arange(NT)[:, None] + np.arange(8)[None, :]
    xp = xpad[:, :, ridx, :].transpose(0, 2, 3, 1, 4).reshape(B, NT, 128, XW)
    ypad = np.zeros((B, 5, SB * NT + 8, W), F8)
    ypad[:, :, 1:H + 1, :] = y8
    yhp = ypad[:, :, ridx, :].transpose(0, 2, 3, 1, 4).reshape(B, NT, 40, W)
    cidx = SB * np.arange(NT)[:, None] + 1 + np.arange(SB)[None, :]
    ycp = ypad[:, :, cidx, :].transpose(0, 2, 3, 1, 4).reshape(B, NT, 30, W)
    return (np.ascontiguousarray(xp), np.ascontiguousarray(yhp),
            np.ascontiguousarray(ycp))


# ---------------------------------------------------------------------------
# bass graph
# ---------------------------------------------------------------------------

def _pairs(v, pair_stride, n, base=0):
    """3-dim DR rhs AP [[part],[pair_stride,2],[1,n]] from a 2-dim view."""
    a = v.unsqueeze(1).copy()
    a.ap[1] = [pair_stride, 2]
    a.ap[2] = [1, n]
    return a


def _build():
    import concourse.bacc as bacc
    import concourse.tile as tile
    from concourse import mybir

    f32 = mybir.dt.float32
    bf16 = mybir.dt.bfloat16
    f8 = mybir.dt.float8e4
    AF = mybir.ActivationFunctionType
    ALU = mybir.AluOpType
    DR = mybir.MatmulPerfMode.DoubleRow

    def _pick(act_cost, dve_cost):
        bal = _cache.setdefault("_bal", [0.0, 0.0])
        if (bal[0] + act_cost) * ACT_W <= bal[1] + dve_cost:
            bal[0] += act_cost
            return 0
        bal[1] += dve_cost
        return 1

    nc = bacc.Bacc("TRN2", target_bir_lowering=False)
    xp_ext = nc.declare_dram_parameter("xp", [NT, 128, XW], f8, isOutput=False)
    yhp_ext = nc.declare_dram_parameter("yhp", [NT, 40, W], f8, isOutput=False)
    ycp_ext = nc.declare_dram_parameter("ycp", [NT, 30, W], f8, isOutput=False)
    wc8_ext = nc.declare_dram_parameter("wc8", [128, W8TOT], f8, isOutput=False)
    wcb_ext = nc.declare_dram_parameter("wcb", [128, WBTOT], bf16,
                                        isOutput=False)
    fcol_ext = nc.declare_dram_parameter("fcol", [128, FCOLS], f32,
                                         isOutput=False)
    out_ext = nc.declare_dram_parameter("out", [NT, 96, W], bf16,
                                        isOutput=True)

    with tile.TileContext(nc) as tc:
        with (
            tc.tile_pool(name="singles", bufs=1) as singles,
            tc.tile_pool(name="ps_fuse", bufs=2, space="PSUM") as ps_fuse,
            tc.tile_pool(name="ps_conv", bufs=2, space="PSUM") as ps_conv,
            tc.tile_pool(name="ps_lc", bufs=2, space="PSUM") as ps_lc,
        ):
            # ---------------- static tiles + warmup -----------------------
            wtile = singles.tile([128, 256], bf16, tag="wtile")
            nc.vector.memset(wtile[:, :], 0.0)
            wps = ps_conv.tile([96, 1024], f32, tag="conv", name="warmps")
            for i in range(N_WARM):
                nc.tensor.matmul(wps[0:96, 0:256], lhsT=wtile[:, 0:96],
                                 rhs=wtile[:, :], start=(i == 0),
                                 stop=(i == N_WARM - 1))

            wc8 = singles.tile([128, W8TOT], f8, tag="wc8")
            nc.sync.dma_start(out=wc8[:, :], in_=wc8_ext[:, :])
            wcb = singles.tile([128, WBTOT], bf16, tag="wcb")
            fcol = singles.tile([128, FCOLS], f32, tag="fcol")
            nc.sync.dma_start(out=fcol[:, :], in_=fcol_ext[:, :])

            def conv_lhsT(d):
                a = wc8[:, 192 * d:192 * (d + 1)].unsqueeze(1).copy()
                a.ap[1] = [96, 2]
                a.ap[2] = [1, 96]
                return a

            def fuse_lhsT(v):
                a = wc8[0:40, C_FUSE + 256 * v:C_FUSE + 256 * (v + 1)] \
                    .unsqueeze(1).copy()
                a.ap[1] = [128, 2]
                a.ap[2] = [1, 128]
                return a

            def lbms_lhsT(g):
                a = wc8[0:126, C_LBMS + 256 * g:C_LBMS + 256 * (g + 1)] \
                    .unsqueeze(1).copy()
                a.ap[1] = [128, 2]
                a.ap[2] = [1, 128]
                return a

            SELW = wcb[:, B_SEL:B_SEL + 96]
            W1L = wcb[:, B_W1:B_W1 + 96]
            W2R = wcb[:, B_W2:B_W2 + 128]

            # ---------------- data tiles ----------------------------------
            xf = singles.tile([128, NT * PW], f8, tag="xf")
            yh = singles.tile([40, RING_YH * XW], f8, tag="yh")
            fcc = singles.tile([128, NT * FW], f8, tag="fcc")
            og = [singles.tile([96, 8 * W], bf16, tag=f"og{k}", name=f"og{k}")
                  for k in (0, 1, 2)]
            sg = [singles.tile([112, W], bf16, tag=f"sg{k}", name=f"sg{k}")
                  for k in (0, 1)]
            sgB = [singles.tile([48, W], bf16, tag=f"sgB{k}", name=f"sgB{k}")
                   for k in (0, 1)]
            svg = [singles.tile([49, W], bf16, tag=f"svg{k}", name=f"svg{k}")
                   for k in (0, 1, 2)]
            rep = [singles.tile([96, W], bf16, tag=f"rep{k}", name=f"rep{k}")
                   for k in (0, 1, 2, 3)]
            Ra = singles.tile([128, NT], f32, tag="Ra")
            nc.vector.memset(Ra[:, :], 0.0)
            Rb = singles.tile([128, NT], f32, tag="Rb")
            nc.vector.memset(Rb[:, :], 0.0)

            # presets: XF F-block pad cols (t=0,513 per ring slot); yh pad
            # cols 512/513 per slot; fcc pad cols 512/513 per slot.
            m = xf[:, XW:XW + 1].unsqueeze(1).copy()
            m.ap[1] = [PW, NT]
            m.ap[2] = [XW - 1, 2]
            nc.gpsimd.memset(m, 0.0)
            m = yh[:, W:W + 1].unsqueeze(1).copy()
            m.ap[1] = [XW, RING_YH]
            m.ap[2] = [1, 2]
            nc.gpsimd.memset(m, 0.0)
            m = fcc[:, W:W + 1].unsqueeze(1).copy()
            m.ap[1] = [FW, NT]
            m.ap[2] = [1, 2]
            nc.gpsimd.memset(m, 0.0)

            # ---------------- phase A + fronts ----------------------------
            def xslot(t):
                return xf[:, t * PW:t * PW + PW]

            def issue_fuse(t):
                if t % 8 == 0:
                    n = min(8, NT - t)
                    s1 = (t % RING_YH)
                    nc.sync.dma_start(
                        out=yh[:, s1 * XW:(s1 + n) * XW].rearrange(
                            "p (s c) -> p s c", s=n)[:, :, 0:W],
                        in_=yhp_ext[t:t + n, :, :].rearrange("s p j -> p s j"))
                    nc.sync.dma_start(
                        out=xf[:, t * PW:(t + n) * PW].rearrange(
                            "p (s c) -> p s c", s=n)[:, :, 0:XW],
                        in_=xp_ext[t:t + n, :, :].rearrange("s p j -> p s j"))
                yv = yh[0:40, (t % RING_YH) * XW:(t % RING_YH) * XW + W]
                fps = ps_fuse.tile([128, 512], f32, tag="fuse")
                v = 1 if t == 0 else (2 if t == NT - 1 else 0)
                nc.tensor.matmul(fps[:, :], lhsT=fuse_lhsT(v),
                                 rhs=_pairs(yv, 2, W), start=True, stop=True,
                                 perf_mode=DR)
                fdst = xslot(t)[:, XW + 1:XW + 1 + W]
                bv = 4 if 0 < t < NT - 1 else (5 if t == 0 else 6)
                bias = fcol[:, bv:bv + 1]
                if _pick(612, 658) == 0:
                    nc.scalar.activation(out=fdst, in_=fps[:, :], func=AF.Relu,
                                         bias=bias, accum_out=Ra[:, t:t + 1])
                else:
                    nc.vector.tensor_scalar(out=fdst, in0=fps[:, :],
                                            scalar1=bias, scalar2=0.0,
                                            op0=ALU.add, op1=ALU.max,
                                            accum_out=Rb[:, t:t + 1])

            cpair = {}

            def issue_front(s):
                xv = xslot(s)
                CD = CONV_DEPTH
                if s % CD == 0:
                    cpair[0] = ps_conv.tile([96, CD * 512], f32, tag="conv",
                                            name="cps")
                cps = cpair[0]
                half = (s % CD) * 512
                for d in range(3):
                    nc.tensor.matmul(cps[:, half:half + 512],
                                     lhsT=conv_lhsT(d),
                                     rhs=_pairs(xv[:, d:], XW, W),
                                     start=(d == 0), stop=(d == 2),
                                     perf_mode=DR)
                nb = s % CD + 1
                if s % CD != CD - 1 and s != NT - 1:
                    return
                s0 = s - nb + 1
                dst = fcc[0:96, s0 * FW:s0 * FW + 1].unsqueeze(1).copy()
                dst.ap[1] = [FW, nb]
                dst.ap[2] = [1, W]
                src = cps[:, 0:nb * 512]
                if _pick(185 + nb * 427, 125 + nb * 533) == 0:
                    nc.scalar.activation(out=dst, in_=src, func=AF.Relu,
                                         bias=fcol[0:96, 0:1])
                else:
                    nc.vector.tensor_scalar(out=dst, in0=src,
                                            scalar1=fcol[0:96, 0:1],
                                            scalar2=0.0,
                                            op0=ALU.add, op1=ALU.max)

            for t in range(NT):
                if t == 2:
                    nc.sync.dma_start(out=wcb[:, :], in_=wcb_ext[:, :])
                    for k in (0, 1, 2):
                        nc.sync.dma_start(
                            out=svg[k][48:49, :],
                            in_=wcb[48:49, B_ONES:B_ONES + 512])
                issue_fuse(t)

            # ---------------- SE chain ------------------------------------
            R_bf = singles.tile([128, NT], bf16, tag="Rbf")
            nc.vector.tensor_add(out=R_bf[:, :], in0=Ra[:, :], in1=Rb[:, :])
            gps = ps_fuse.tile([96, NT], f32, tag="fuse")
            nc.tensor.matmul(gps[:, :], lhsT=SELW, rhs=R_bf[:, :],
                             start=True, stop=True)
            gap_f = singles.tile([96, 1], f32, tag="gapf")
            nc.vector.reduce_sum(out=gap_f[:, :], in_=gps[:, :],
                                 axis=mybir.AxisListType.X)
            gap_bf = singles.tile([128, 1], bf16, tag="gap")
            nc.vector.memset(gap_bf[:, :], 0.0)
            nc.vector.tensor_copy(out=gap_bf[0:96, :], in_=gap_f[:, :])
            hps = ps_fuse.tile([96, 1], f32, tag="fuse")
            nc.tensor.matmul(hps[:, :], lhsT=W1L, rhs=gap_bf[:, :],
                             start=True, stop=True)
            h_bf = singles.tile([128, 1], bf16, tag="hbf")
            nc.vector.memset(h_bf[:, :], 0.0)
            nc.scalar.activation(out=h_bf[0:96, :], in_=hps[:, :], func=AF.Relu)
            sps = ps_fuse.tile([128, 1], f32, tag="fuse")
            nc.tensor.matmul(sps[:, :], lhsT=W2R, rhs=h_bf[:, :],
                             start=True, stop=True)
            se_bc = singles.tile([128, 1], f32, tag="sebc")
            nc.scalar.activation(out=se_bc[:, :], in_=sps[:, :],
                                 func=AF.Sigmoid)
            # fill LBMS bd rows (96..125): wc8 <- PSBW * se
            nc.scalar.activation(out=wc8[96:126, C_LBMS:C_LBMS + 1024],
                                 in_=wcb[96:126, B_PSBW:B_PSBW + 1024],
                                 func=AF.Copy, scale=se_bc[96:126, :])
            nc.vector.tensor_scalar(
                out=wc8[96:126, C_LBMS + 1024:C_LBMS + 2048],
                in0=wcb[96:126, B_PSBW + 1024:B_PSBW + 2048],
                scalar1=se_bc[96:126, :], scalar2=0.0,
                op0=ALU.mult, op1=ALU.add)

            # ---------------- tails ---------------------------------------
            def issue_tail(u):
                g = u % 8
                G = u // 8
                gp = _cache.setdefault("_gp", {})
                if g == 0:
                    gp[G] = ps_fuse.tile([128, 512], f32, tag="fuse",
                                         name="grp")
                GP = gp[G]
                fv = fcc[0:126, u * FW:u * FW + W]
                nc.tensor.matmul(GP[:, :], lhsT=lbms_lhsT(g),
                                 rhs=_pairs(fv, 2, W), start=(g == 0),
                                 stop=(g == 7 or u == NT - 1), perf_mode=DR)
            def issue_chain(G):
                u = min(8 * G + 7, NT - 1)
                ng = u - 8 * G + 1           # strips in this group
                gp = _cache.setdefault("_gp", {})
                GP = gp[G]
                sgt, svt = sg[G % 2], svg[G % 3]
                np_ = 64 + 6 * ng
                bal = _cache.setdefault("_bal", [0.0, 0.0])
                bal[0] += 612.0
                nc.scalar.activation(out=sgt[0:np_, :], in_=GP[0:np_, :],
                                     func=AF.Sigmoid, bias=fcol[0:np_, 3:4])
                sbt = sgB[G % 2]
                ce = nc.gpsimd if CHAIN_POOL else nc.vector
                if not CHAIN_POOL:
                    bal[1] += 713.0
                ce.tensor_copy(out=sbt[0:6 * ng, :],
                               in_=sgt[64:64 + 6 * ng, :])
                ce.tensor_add(out=svt[0:6 * ng, :],
                              in0=sgt[0:6 * ng, :],
                              in1=sbt[0:6 * ng, :])
                ce.tensor_scalar(out=svt[0:6 * ng, :],
                                 in0=svt[0:6 * ng, :],
                                 scalar1=1.0, scalar2=0.0,
                                 op0=ALU.min, op1=ALU.add)

            def og_dma(G, half=None):
                ng = min(8, NT - 8 * G)
                lo, hi = 0, ng
                if half == 0:
                    hi = min(4, ng)
                elif half == 1:
                    lo = 4
                    if ng <= 4:
                        return
                nc.sync.dma_start(
                    out=out_ext[8 * G + lo:8 * G + hi, :, :].rearrange(
                        "s p j -> p s j"),
                    in_=og[G % 3][0:96, lo * W:hi * W])

            def issue_final(uu):
                G = uu // 8
                gg = uu % 8
                svt = svg[G % 3]
                if True:
                    dst = og[G % 3][:, gg * W:gg * W + W]
                    if _is_rep(uu):
                        rt = rep[_cache.setdefault('_rr', [0])[0] % 4]
                        _cache['_rr'][0] += 1
                        src = svt[6 * gg:6 * gg + 6, :].unsqueeze(1).copy()
                        src.ap[1] = [0, 16]
                        nc.sync.dma_start(out=rt[:, :], in_=src)
                        k = _cache.setdefault("_repn", [0])
                        k[0] += 1
                        eng = nc.gpsimd if k[0] % 2 == 0 else nc.vector
                        eng.tensor_scalar(out=dst, in0=rt[:, :],
                                          scalar1=fcol[0:96, 1:2],
                                          scalar2=fcol[0:96, 2:3],
                                          op0=ALU.mult, op1=ALU.add)
                    else:
                        ops = ps_lc.tile([96, 512], f32, tag="lc")
                        nc.tensor.matmul(ops[:, :],
                                         lhsT=wcb[0:49, B_LCW + 96 * gg:
                                                  B_LCW + 96 * (gg + 1)],
                                         rhs=svt[0:49, :],
                                         start=True, stop=True)
                        if _pick(612, 658) == 0:
                            nc.scalar.activation(out=dst, in_=ops[:, :],
                                                 func=AF.Copy)
                        else:
                            nc.vector.tensor_copy(out=dst, in_=ops[:, :])

            for s0 in range(0, NT, 8):
                n = min(8, NT - s0)
                nc.sync.dma_start(
                    out=fcc[96:126, s0 * FW:(s0 + n) * FW].rearrange(
                        "p (s c) -> p s c", s=n)[:, :, 0:W],
                    in_=ycp_ext[s0:s0 + n, :, :].rearrange("s p j -> p s j"))
            tc = [0]
            fc_ = [0]
            cc = [0]
            vu = [-1]
            s = 0
            while fc_[0] < NT:
                if s < NT:
                    issue_front(s)
                pace = 1 if s < NT else 2
                if s >= TAIL_LAG:
                    vu[0] += pace
                for _ in range(pace):
                    if tc[0] < NT and tc[0] <= vu[0]:
                        issue_tail(tc[0])
                        tc[0] += 1
                while cc[0] < NG and vu[0] - min(8 * cc[0] + 7, NT - 1) \
                        >= CHAIN_LAG:
                    issue_chain(cc[0])
                    cc[0] += 1
                uf_max = vu[0] - CHAIN_LAG - FIN_LAG
                nfin = pace
                while fc_[0] <= uf_max and fc_[0] < NT and nfin > 0 \
                        and fc_[0] // 8 < cc[0]:
                    uf = fc_[0]
                    fc_[0] += 1
                    nfin -= 1
                    issue_final(uf)
                    if uf % 8 == 7 or uf == NT - 1:
                        og_dma(uf // 8)
                s += 1
    nc.compile()
    for k in ("_gp", "_lcn", "_repn", "_bal", "_rr", "_rq", "_fc", "_cc"):
        _cache.pop(k, None)
    return nc


# ---------------------------------------------------------------------------
# entry point
# ---------------------------------------------------------------------------

LAST_RESULT = None


def prepare(x, y, fuse_w, fuse_b, se_w1, se_w2, bd_w, bd_b,
            fc_w, fc_b, fm_w, fm_b, cv_w, cv_b):
    if "nc" not in _cache:
        _cache["nc"] = _build()
    nc = _cache["nc"]
    g = {k: np.asarray(v, np.float32) for k, v in dict(
        fuse_w=fuse_w, fuse_b=fuse_b, se_w1=se_w1, se_w2=se_w2, bd_w=bd_w,
        bd_b=bd_b, fc_w=fc_w, fc_b=fc_b, fm_w=fm_w, fm_b=fm_b, cv_w=cv_w,
        cv_b=cv_b).items()}
    wc8 = np.zeros((128, W8TOT), F8)
    wc8[:, 0:576] = _conv_w8(g["fc_w"])
    wc8[0:40, C_FUSE:C_FUSE + 768] = _fuse_w8(g["fuse_w"])
    wc8[:, C_LBMS:C_LBMS + 2048] = _lbms_w8(g["fm_w"])
    wcb = _wcb(g["cv_w"], g["cv_b"], g["se_w1"], g["se_w2"], g["bd_w"])
    fcol = _fcol(g["fc_b"], g["cv_w"], g["cv_b"], g["fm_b"], g["bd_b"],
                 g["fuse_b"])
    xb = np.asarray(x, np.float32)
    yb = np.asarray(y, np.float32)
    xp, yhp, ycp = _pack_inputs(xb, yb)
    in_maps = [
        {"xp": xp[i], "yhp": yhp[i], "ycp": ycp[i],
         "wc8": wc8, "wcb": wcb, "fcol": fcol}
        for i in range(xb.shape[0])
    ]
    return nc, in_maps


def kernel(x, y, fuse_w, fuse_b, se_w1, se_w2, bd_w, bd_b,
           fc_w, fc_b, fm_w, fm_b, cv_w, cv_b):
    global LAST_RESULT
    from concourse.bass_utils import run_bass_kernel_spmd

    nc, in_maps = prepare(x, y, fuse_w, fuse_b, se_w1, se_w2, bd_w, bd_b,
                          fc_w, fc_b, fm_w, fm_b, cv_w, cv_b)
    res = run_bass_kernel_spmd(nc, in_maps, core_ids=list(range(8)))
    LAST_RESULT = res
    outs = []
    for i in range(len(in_maps)):
        ot = np.asarray(res.results[i]["out"], np.float32)   # [NT, 96, W]
        full = ot.reshape(NT, SB, 16, W).transpose(2, 0, 1, 3) \
                 .reshape(16, NT * SB, W)[:, :H, :]
        outs.append(full)
    return np.stack(outs)


# revision 8
# speedup vs baseline: 1.0473x; 1.0109x over previous
"""Trainium2 Bass kernel for nn_Boundary_Enchance (dense_cnn), v2.

Pure data parallel: core i processes batch image i.  Compute is fp8-e4m3
DoubleRow on the PE for the fuse 1x1, the 3x3 conv, and the merged
mask+boundary head (validated rel-err 0.004 on host); the final 1->16
expansion stays bf16 (fp8 weights there cost ~2% systematic error).

Layout tricks:
  - XF tiles [128, 1028] fp8: cols 0..513 = x (8 rows x 16ch, image cols
    -1..512, host-packed), cols 514..1027 = F = relu(fuse(y)) written by the
    fuse evacuation.  The 3x3 conv is THREE DoubleRow matmuls (one per dx
    tap): the DR pair (stride 514) contracts x and F simultaneously, K_eff
    = 256 = 8 rows x 32 ch.  dy taps ride the row-Toeplitz lhsT; dx taps
    are rhs base-column shifts.
  - fuse: one DR matmul per 8-row tile, pair (j, j+2) with zero second
    weights; bias folded into the evacuation tensor_scalar (per-tile bias
    column variants handle the image edges).
  - mask+boundary (LBM): per strip one DR matmul with M=128 accumulating 8
    strips into ONE psum bank (strip g -> mask logits at partitions 6g+i,
    boundary at 64+6g+i, zeros elsewhere).  Tail nonlinearities run ONCE
    per 8 strips: sigmoid (with per-partition bias = head biases), add,
    min -> sv group tile [48, 512].
  - final expansion per strip: either an LC bf16 matmul + psum evacuation,
    or a replicating SBUF DMA + per-partition affine (x cv_w + cv_b) on
    the DVE at 4x rate ("REP path") -- mixed to balance PE/DMA/engines.
"""

import numpy as np
import ml_dtypes

BF16 = ml_dtypes.bfloat16
F8 = ml_dtypes.float8_e4m3

H = 512
W = 512
SB = 6
NT = (H + SB - 1) // SB          # 86 strips
XW = 514                          # x / F block width (image cols -1..512)
PW = 2 * XW                       # XF tile width
FW = 514                          # fcc slot width (512 + 2 pad)
NG = (NT + 7) // 8                # strip groups of 8

RING_YH = 16
TAIL_LAG = 8                      # tails lag fronts in phase B
N_WARM = 20
CHAIN_POOL = False


REP_FRAC = 3                      # u % REP_FRAC == 1 -> REP; 0 disables
CONV_DEPTH = 2                    # strips per conv psum tile group
CHAIN_LAG = 3
FIN_LAG = 8
ACT_W = 0.97
BAL_DECAY = 1.0
FUSE_BUFS = 2
LC_BUFS = 2
AFF_MOD = 1000000


REP_END = 72                      # strips >= REP_END never use REP


REP_MID = 0                       # strips in [REP_MID, REP_END) use u%2==1
OG_SPLIT = False


def _is_rep(u):
    if u >= REP_END:
        return False
    if REP_MID and u >= REP_MID:
        return (u % 2) == 1
    return REP_FRAC == 1 or (REP_FRAC > 0 and (u % REP_FRAC) == 1)

# wc8 fp8 column map
C_CONV = 0                        # 3 taps x [2,96] = 576
C_FUSE = 576                      # 3 variants x [2,128] = 768
C_LBMS = 576 + 768                # 8 variants x [2,128] = 2048
W8TOT = C_LBMS + 2048
# wcb bf16 column map
B_LCW = 0                         # 8 variants x 96 = 768
B_SEL = 768                       # 96
B_W1 = 768 + 96                   # 96
B_W2 = 768 + 192                  # 128
B_PSBW = 768 + 320                # 2048 (partitions 96..125)
B_ONES = B_PSBW + 2048            # 512 cols of 1.0 at partition 48
WBTOT = B_ONES + 512
# fcol f32 columns: 0 fcb, 1 cvw, 2 cvb, 3 sigbias, 4..6 fuse bias variants
FCOLS = 7

_cache = {}


# ---------------------------------------------------------------------------
# host-side packing
# ---------------------------------------------------------------------------

def _conv_w8(fc_w):
    """[128, 576] fp8: tap d at cols 192d..192d+191, pair p block of 96:
    w[k=r*16+c, 192d+96p+i*16+oc] = fc_w[oc, 16p+c, r-i, d] for r-i in 0..2."""
    out = np.zeros((128, 576), np.float32)
    for d in range(3):
        for p in range(2):
            for i in range(SB):
                for ky in range(3):
                    r = i + ky
                    out[r * 16:r * 16 + 16,
                        192 * d + 96 * p + i * 16:192 * d + 96 * p + i * 16 + 16] = \
                        fc_w[:, 16 * p:16 * p + 16, ky, d].T
    return out.astype(F8)


def _fuse_w8(fuse_w):
    """[40, 768] fp8: 3 identical variants (edges handled by evac bias cols):
    w[k=r*5+yc, 256v+0*128+r*16+oc] = fuse_w[oc, yc]; pair1 block zero."""
    out = np.zeros((40, 768), np.float32)
    for v in range(3):
        for r in range(8):
            out[r * 5:r * 5 + 5, 256 * v + r * 16:256 * v + r * 16 + 16] = \
                fuse_w[:, :, 0, 0].T
    return out.astype(F8)


def _lbms_w8(fm_w):
    """[128, 2048] fp8 static part: variant g at cols 256g..: pair0
    [k=i*16+c, m=6g+i] = fm diff; rows 96..125 (bd) filled on device."""
    out = np.zeros((128, 2048), np.float32)
    d = fm_w[1, :, 0, 0] - fm_w[0, :, 0, 0]
    for g in range(8):
        for i in range(SB):
            out[i * 16:i * 16 + 16, 256 * g + 6 * g + i] = d
    return out.astype(F8)


def _wcb(cv_w, cv_b, se_w1, se_w2, bd_w):
    out = np.zeros((128, WBTOT), np.float32)
    # LCW: 8 variants [49 rows, 96]: rows 6g+i -> cv_w; row 48 (ones) -> cv_b
    for g in range(8):
        for i in range(SB):
            out[6 * g + i, B_LCW + 96 * g + i * 16:B_LCW + 96 * g + i * 16 + 16] = \
                cv_w[:, 0, 0, 0]
        for i in range(SB):
            out[48, B_LCW + 96 * g + i * 16:B_LCW + 96 * g + i * 16 + 16] = cv_b
    out[48, B_ONES:B_ONES + 512] = 1.0
    # SEL: [r*16+oc, oc] = 1/NPIX for r in 1..6
    for r in range(1, 7):
        for fc in range(16):
            out[r * 16 + fc, B_SEL + fc] = 1.0 / float(H * W)
    # W1
    out[0:16, B_W1:B_W1 + 16] = se_w1.T
    # W2: cols 96 + r*5 + yc <- se_w2.T (targets se_bc partitions 96..125)
    for r in range(SB):
        out[0:16, B_W2 + 96 + r * 5:B_W2 + 96 + r * 5 + 5] = se_w2.T
    # PSBW: partitions 96..125: [96 + r*5+yc, 256g + 64+6g+r] = bd diff
    dbd = bd_w[1, :, 0, 0] - bd_w[0, :, 0, 0]
    for g in range(8):
        for r in range(SB):
            out[96 + r * 5:96 + r * 5 + 5,
                B_PSBW + 256 * g + 64 + 6 * g + r] = dbd
    return out.astype(BF16)


def _fcol(fc_b, cv_w, cv_b, fm_b, bd_b, fuse_b):
    out = np.zeros((128, FCOLS), np.float32)
    for i in range(SB):
        out[i * 16:(i + 1) * 16, 0] = fc_b          # conv bias (+relu evac)
        out[i * 16:(i + 1) * 16, 1] = cv_w[:, 0, 0, 0]
        out[i * 16:(i + 1) * 16, 2] = cv_b
    for g in range(8):
        out[6 * g:6 * g + 6, 3] = fm_b[1] - fm_b[0]      # sigmoid bias: mask
        out[64 + 6 * g:64 + 6 * g + 6, 3] = bd_b[1] - bd_b[0]  # boundary
    for r in range(8):
        out[r * 16:r * 16 + 16, 4] = fuse_b          # interior fuse bias
        out[r * 16:r * 16 + 16, 5] = fuse_b if r > 0 else 0.0   # first tile
        out[r * 16:r * 16 + 16, 6] = fuse_b if r < 3 else 0.0   # last tile
    return out


def _pack_inputs(xb, yb):
    """xp [NT,128,514] fp8; yhp [NT,40,512] fp8; ycp [NT,30,512] fp8."""
    B = xb.shape[0]
    x8 = xb.astype(F8)
    y8 = yb.astype(F8)
    xpad = np.zeros((B, 16, SB * NT + 8, W + 2), F8)
    xpad[:, :, 1:H + 1, 1:W + 1] = x8
    ridx = SB * np.arange(NT)[:, None] + np.arange(8)[None, :]
    xp = xpad[:, :, ridx, :].transpose(0, 2, 3, 1, 4).reshape(B, NT, 128, XW)
    ypad = np.zeros((B, 5, SB * NT + 8, W), F8)
    ypad[:, :, 1:H + 1, :] = y8
    yhp = ypad[:, :, ridx, :].transpose(0, 2, 3, 1, 4).reshape(B, NT, 40, W)
    cidx = SB * np.arange(NT)[:, None] + 1 + np.arange(SB)[None, :]
    ycp = ypad[:, :, cidx, :].transpose(0, 2, 3, 1, 4).reshape(B, NT, 30, W)
    return (np.ascontiguousarray(xp), np.ascontiguousarray(yhp),
            np.ascontiguousarray(ycp))


# ---------------------------------------------------------------------------
# bass graph
# ---------------------------------------------------------------------------

def _pairs(v, pair_stride, n, base=0):
    """3-dim DR rhs AP [[part],[pair_stride,2],[1,n]] from a 2-dim view."""
    a = v.unsqueeze(1).copy()
    a.ap[1] = [pair_stride, 2]
    a.ap[2] = [1, n]
    return a


def _build():
    import concourse.bacc as bacc
    import concourse.tile as tile
    from concourse import mybir

    f32 = mybir.dt.float32
    bf16 = mybir.dt.bfloat16
    f8 = mybir.dt.float8e4
    AF = mybir.ActivationFunctionType
    ALU = mybir.AluOpType
    DR = mybir.MatmulPerfMode.DoubleRow

    def _pick(act_cost, dve_cost):
        bal = _cache.setdefault("_bal", [0.0, 0.0])
        bal[0] *= BAL_DECAY
        bal[1] *= BAL_DECAY
        if (bal[0] + act_cost) * ACT_W <= bal[1] + dve_cost:
            bal[0] += act_cost
            return 0
        bal[1] += dve_cost
        return 1

    nc = bacc.Bacc("TRN2", target_bir_lowering=False)
    xp_ext = nc.declare_dram_parameter("xp", [NT, 128, XW], f8, isOutput=False)
    yhp_ext = nc.declare_dram_parameter("yhp", [NT, 40, W], f8, isOutput=False)
    ycp_ext = nc.declare_dram_parameter("ycp", [NT, 30, W], f8, isOutput=False)
    wc8_ext = nc.declare_dram_parameter("wc8", [128, W8TOT], f8, isOutput=False)
    wcb_ext = nc.declare_dram_parameter("wcb", [128, WBTOT], bf16,
                                        isOutput=False)
    fcol_ext = nc.declare_dram_parameter("fcol", [128, FCOLS], f32,
                                         isOutput=False)
    out_ext = nc.declare_dram_parameter("out", [NT, 96, W], bf16,
                                        isOutput=True)

    with tile.TileContext(nc) as tc:
        with (
            tc.tile_pool(name="singles", bufs=1) as singles,
            tc.tile_pool(name="ps_fuse", bufs=FUSE_BUFS, space="PSUM")
                as ps_fuse,
            tc.tile_pool(name="ps_conv", bufs=2, space="PSUM") as ps_conv,
            tc.tile_pool(name="ps_lc", bufs=LC_BUFS, space="PSUM") as ps_lc,
        ):
            # ---------------- static tiles + warmup -----------------------
            wtile = singles.tile([128, 256], bf16, tag="wtile")
            nc.vector.memset(wtile[:, :], 0.0)
            wps = ps_conv.tile([96, 1024], f32, tag="conv", name="warmps")
            for i in range(N_WARM):
                nc.tensor.matmul(wps[0:96, 0:256], lhsT=wtile[:, 0:96],
                                 rhs=wtile[:, :], start=(i == 0),
                                 stop=(i == N_WARM - 1))

            wc8 = singles.tile([128, W8TOT], f8, tag="wc8")
            nc.sync.dma_start(out=wc8[:, :], in_=wc8_ext[:, :])
            wcb = singles.tile([128, WBTOT], bf16, tag="wcb")
            fcol = singles.tile([128, FCOLS], f32, tag="fcol")
            nc.sync.dma_start(out=fcol[:, :], in_=fcol_ext[:, :])

            def conv_lhsT(d):
                a = wc8[:, 192 * d:192 * (d + 1)].unsqueeze(1).copy()
                a.ap[1] = [96, 2]
                a.ap[2] = [1, 96]
                return a

            def fuse_lhsT(v):
                a = wc8[0:40, C_FUSE + 256 * v:C_FUSE + 256 * (v + 1)] \
                    .unsqueeze(1).copy()
                a.ap[1] = [128, 2]
                a.ap[2] = [1, 128]
                return a

            def lbms_lhsT(g):
                a = wc8[0:126, C_LBMS + 256 * g:C_LBMS + 256 * (g + 1)] \
                    .unsqueeze(1).copy()
                a.ap[1] = [128, 2]
                a.ap[2] = [1, 128]
                return a

            SELW = wcb[:, B_SEL:B_SEL + 96]
            W1L = wcb[:, B_W1:B_W1 + 96]
            W2R = wcb[:, B_W2:B_W2 + 128]

            # ---------------- data tiles ----------------------------------
            xf = singles.tile([128, NT * PW], f8, tag="xf")
            yh = singles.tile([40, RING_YH * XW], f8, tag="yh")
            fcc = singles.tile([128, NT * FW], f8, tag="fcc")
            og = [singles.tile([96, 8 * W], bf16, tag=f"og{k}", name=f"og{k}")
                  for k in (0, 1, 2)]
            sg = [singles.tile([112, W], bf16, tag=f"sg{k}", name=f"sg{k}")
                  for k in (0, 1)]
            sgB = [singles.tile([48, W], bf16, tag=f"sgB{k}", name=f"sgB{k}")
                   for k in (0, 1)]
            svg = [singles.tile([49, W], bf16, tag=f"svg{k}", name=f"svg{k}")
                   for k in (0, 1, 2)]
            rep = [singles.tile([96, W], bf16, tag=f"rep{k}", name=f"rep{k}")
                   for k in (0, 1, 2, 3)]
            Ra = singles.tile([128, NT], f32, tag="Ra")
            nc.vector.memset(Ra[:, :], 0.0)
            Rb = singles.tile([128, NT], f32, tag="Rb")
            nc.vector.memset(Rb[:, :], 0.0)

            # presets: XF F-block pad cols (t=0,513 per ring slot); yh pad
            # cols 512/513 per slot; fcc pad cols 512/513 per slot.
            m = xf[:, XW:XW + 1].unsqueeze(1).copy()
            m.ap[1] = [PW, NT]
            m.ap[2] = [XW - 1, 2]
            nc.gpsimd.memset(m, 0.0)
            m = yh[:, W:W + 1].unsqueeze(1).copy()
            m.ap[1] = [XW, RING_YH]
            m.ap[2] = [1, 2]
            nc.gpsimd.memset(m, 0.0)
            m = fcc[:, W:W + 1].unsqueeze(1).copy()
            m.ap[1] = [FW, NT]
            m.ap[2] = [1, 2]
            nc.gpsimd.memset(m, 0.0)

            # ---------------- phase A + fronts ----------------------------
            def xslot(t):
                return xf[:, t * PW:t * PW + PW]

            def issue_fuse(t):
                if t % 8 == 0:
                    n = min(8, NT - t)
                    s1 = (t % RING_YH)
                    nc.sync.dma_start(
                        out=yh[:, s1 * XW:(s1 + n) * XW].rearrange(
                            "p (s c) -> p s c", s=n)[:, :, 0:W],
                        in_=yhp_ext[t:t + n, :, :].rearrange("s p j -> p s j"))
                    nc.sync.dma_start(
                        out=xf[:, t * PW:(t + n) * PW].rearrange(
                            "p (s c) -> p s c", s=n)[:, :, 0:XW],
                        in_=xp_ext[t:t + n, :, :].rearrange("s p j -> p s j"))
                yv = yh[0:40, (t % RING_YH) * XW:(t % RING_YH) * XW + W]
                fps = ps_fuse.tile([128, 512], f32, tag="fuse")
                v = 1 if t == 0 else (2 if t == NT - 1 else 0)
                nc.tensor.matmul(fps[:, :], lhsT=fuse_lhsT(v),
                                 rhs=_pairs(yv, 2, W), start=True, stop=True,
                                 perf_mode=DR)
                fdst = xslot(t)[:, XW + 1:XW + 1 + W]
                bv = 4 if 0 < t < NT - 1 else (5 if t == 0 else 6)
                bias = fcol[:, bv:bv + 1]
                if _pick(612, 658) == 0:
                    nc.scalar.activation(out=fdst, in_=fps[:, :], func=AF.Relu,
                                         bias=bias, accum_out=Ra[:, t:t + 1])
                else:
                    nc.vector.tensor_scalar(out=fdst, in0=fps[:, :],
                                            scalar1=bias, scalar2=0.0,
                                            op0=ALU.add, op1=ALU.max,
                                            accum_out=Rb[:, t:t + 1])

            cpair = {}

            def issue_front(s):
                xv = xslot(s)
                CD = CONV_DEPTH
                if s % CD == 0:
                    cpair[0] = ps_conv.tile([96, CD * 512], f32, tag="conv",
                                            name="cps")
                cps = cpair[0]
                half = (s % CD) * 512
                for d in range(3):
                    nc.tensor.matmul(cps[:, half:half + 512],
                                     lhsT=conv_lhsT(d),
                                     rhs=_pairs(xv[:, d:], XW, W),
                                     start=(d == 0), stop=(d == 2),
                                     perf_mode=DR)
                nb = s % CD + 1
                if s % CD != CD - 1 and s != NT - 1:
                    return
                s0 = s - nb + 1
                dst = fcc[0:96, s0 * FW:s0 * FW + 1].unsqueeze(1).copy()
                dst.ap[1] = [FW, nb]
                dst.ap[2] = [1, W]
                src = cps[:, 0:nb * 512]
                if _pick(185 + nb * 427, 125 + nb * 533) == 0:
                    nc.scalar.activation(out=dst, in_=src, func=AF.Relu,
                                         bias=fcol[0:96, 0:1])
                else:
                    nc.vector.tensor_scalar(out=dst, in0=src,
                                            scalar1=fcol[0:96, 0:1],
                                            scalar2=0.0,
                                            op0=ALU.add, op1=ALU.max)

            for t in range(NT):
                if t == 2:
                    nc.sync.dma_start(out=wcb[:, :], in_=wcb_ext[:, :])
                    for k in (0, 1, 2):
                        nc.sync.dma_start(
                            out=svg[k][48:49, :],
                            in_=wcb[48:49, B_ONES:B_ONES + 512])
                issue_fuse(t)

            # ---------------- SE chain ------------------------------------
            R_bf = singles.tile([128, NT], bf16, tag="Rbf")
            nc.vector.tensor_add(out=R_bf[:, :], in0=Ra[:, :], in1=Rb[:, :])
            gps = ps_fuse.tile([96, NT], f32, tag="fuse")
            nc.tensor.matmul(gps[:, :], lhsT=SELW, rhs=R_bf[:, :],
                             start=True, stop=True)
            gap_f = singles.tile([96, 1], f32, tag="gapf")
            nc.vector.reduce_sum(out=gap_f[:, :], in_=gps[:, :],
                                 axis=mybir.AxisListType.X)
            gap_bf = singles.tile([128, 1], bf16, tag="gap")
            nc.vector.memset(gap_bf[:, :], 0.0)
            nc.vector.tensor_copy(out=gap_bf[0:96, :], in_=gap_f[:, :])
            hps = ps_fuse.tile([96, 1], f32, tag="fuse")
            nc.tensor.matmul(hps[:, :], lhsT=W1L, rhs=gap_bf[:, :],
                             start=True, stop=True)
            h_bf = singles.tile([128, 1], bf16, tag="hbf")
            nc.vector.memset(h_bf[:, :], 0.0)
            nc.scalar.activation(out=h_bf[0:96, :], in_=hps[:, :], func=AF.Relu)
            sps = ps_fuse.tile([128, 1], f32, tag="fuse")
            nc.tensor.matmul(sps[:, :], lhsT=W2R, rhs=h_bf[:, :],
                             start=True, stop=True)
            se_bc = singles.tile([128, 1], f32, tag="sebc")
            nc.scalar.activation(out=se_bc[:, :], in_=sps[:, :],
                                 func=AF.Sigmoid)
            # fill LBMS bd rows (96..125): wc8 <- PSBW * se
            nc.scalar.activation(out=wc8[96:126, C_LBMS:C_LBMS + 1024],
                                 in_=wcb[96:126, B_PSBW:B_PSBW + 1024],
                                 func=AF.Copy, scale=se_bc[96:126, :])
            nc.vector.tensor_scalar(
                out=wc8[96:126, C_LBMS + 1024:C_LBMS + 2048],
                in0=wcb[96:126, B_PSBW + 1024:B_PSBW + 2048],
                scalar1=se_bc[96:126, :], scalar2=0.0,
                op0=ALU.mult, op1=ALU.add)

            # ---------------- tails ---------------------------------------
            def issue_tail(u):
                g = u % 8
                G = u // 8
                gp = _cache.setdefault("_gp", {})
                if g == 0:
                    gp[G] = ps_fuse.tile([128, 512], f32, tag="fuse",
                                         name="grp")
                GP = gp[G]
                fv = fcc[0:126, u * FW:u * FW + W]
                nc.tensor.matmul(GP[:, :], lhsT=lbms_lhsT(g),
                                 rhs=_pairs(fv, 2, W), start=(g == 0),
                                 stop=(g == 7 or u == NT - 1), perf_mode=DR)
            def issue_chain(G):
                u = min(8 * G + 7, NT - 1)
                ng = u - 8 * G + 1           # strips in this group
                gp = _cache.setdefault("_gp", {})
                GP = gp[G]
                sgt, svt = sg[G % 2], svg[G % 3]
                np_ = 64 + 6 * ng
                bal = _cache.setdefault("_bal", [0.0, 0.0])
                bal[0] += 612.0
                nc.scalar.activation(out=sgt[0:np_, :], in_=GP[0:np_, :],
                                     func=AF.Sigmoid, bias=fcol[0:np_, 3:4])
                sbt = sgB[G % 2]
                ce = nc.gpsimd if CHAIN_POOL else nc.vector
                if not CHAIN_POOL:
                    bal[1] += 713.0
                ce.tensor_copy(out=sbt[0:6 * ng, :],
                               in_=sgt[64:64 + 6 * ng, :])
                ce.tensor_add(out=svt[0:6 * ng, :],
                              in0=sgt[0:6 * ng, :],
                              in1=sbt[0:6 * ng, :])
                ce.tensor_scalar(out=svt[0:6 * ng, :],
                                 in0=svt[0:6 * ng, :],
                                 scalar1=1.0, scalar2=0.0,
                                 op0=ALU.min, op1=ALU.add)

            def og_dma(G, half=None):
                ng = min(8, NT - 8 * G)
                lo, hi = 0, ng
                if half == 0:
                    hi = min(4, ng)
                elif half == 1:
                    lo = 4
                    if ng <= 4:
                        return
                nc.sync.dma_start(
                    out=out_ext[8 * G + lo:8 * G + hi, :, :].rearrange(
                        "s p j -> p s j"),
                    in_=og[G % 3][0:96, lo * W:hi * W])

            def issue_final(uu):
                G = uu // 8
                gg = uu % 8
                svt = svg[G % 3]
                if True:
                    dst = og[G % 3][:, gg * W:gg * W + W]
                    if _is_rep(uu):
                        rt = rep[_cache.setdefault('_rr', [0])[0] % 4]
                        _cache['_rr'][0] += 1
                        src = svt[6 * gg:6 * gg + 6, :].unsqueeze(1).copy()
                        src.ap[1] = [0, 16]
                        nc.sync.dma_start(out=rt[:, :], in_=src)
                        k = _cache.setdefault("_repn", [0])
                        k[0] += 1
                        eng = nc.gpsimd if k[0] % AFF_MOD != 0 else nc.vector
                        eng.tensor_scalar(out=dst, in0=rt[:, :],
                                          scalar1=fcol[0:96, 1:2],
                                          scalar2=fcol[0:96, 2:3],
                                          op0=ALU.mult, op1=ALU.add)
                    else:
                        ops = ps_lc.tile([96, 512], f32, tag="lc")
                        nc.tensor.matmul(ops[:, :],
                                         lhsT=wcb[0:49, B_LCW + 96 * gg:
                                                  B_LCW + 96 * (gg + 1)],
                                         rhs=svt[0:49, :],
                                         start=True, stop=True)
                        if _pick(612, 658) == 0:
                            nc.scalar.activation(out=dst, in_=ops[:, :],
                                                 func=AF.Copy)
                        else:
                            nc.vector.tensor_copy(out=dst, in_=ops[:, :])

            for s0 in range(0, NT, 8):
                n = min(8, NT - s0)
                nc.sync.dma_start(
                    out=fcc[96:126, s0 * FW:(s0 + n) * FW].rearrange(
                        "p (s c) -> p s c", s=n)[:, :, 0:W],
                    in_=ycp_ext[s0:s0 + n, :, :].rearrange("s p j -> p s j"))
            tc = [0]
            fc_ = [0]
            cc = [0]
            vu = [-1]
            s = 0
            while fc_[0] < NT:
                if s < NT:
                    issue_front(s)
                pace = 1 if s < NT else 2
                if s >= TAIL_LAG:
                    vu[0] += pace
                for _ in range(pace):
                    if tc[0] < NT and tc[0] <= vu[0]:
                        issue_tail(tc[0])
                        tc[0] += 1
                while cc[0] < NG and vu[0] - min(8 * cc[0] + 7, NT - 1) \
                        >= CHAIN_LAG:
                    issue_chain(cc[0])
                    cc[0] += 1
                uf_max = vu[0] - CHAIN_LAG - FIN_LAG
                nfin = pace
                while fc_[0] <= uf_max and fc_[0] < NT and nfin > 0 \
                        and fc_[0] // 8 < cc[0]:
                    uf = fc_[0]
                    fc_[0] += 1
                    nfin -= 1
                    issue_final(uf)
                    if uf % 8 == 7 or uf == NT - 1:
                        og_dma(uf // 8)
                s += 1
    nc.compile()
    for k in ("_gp", "_lcn", "_repn", "_bal", "_rr", "_rq", "_fc", "_cc"):
        _cache.pop(k, None)
    return nc


# ---------------------------------------------------------------------------
# entry point
# ---------------------------------------------------------------------------

LAST_RESULT = None


def prepare(x, y, fuse_w, fuse_b, se_w1, se_w2, bd_w, bd_b,
            fc_w, fc_b, fm_w, fm_b, cv_w, cv_b):
    if "nc" not in _cache:
        _cache["nc"] = _build()
    nc = _cache["nc"]
    g = {k: np.asarray(v, np.float32) for k, v in dict(
        fuse_w=fuse_w, fuse_b=fuse_b, se_w1=se_w1, se_w2=se_w2, bd_w=bd_w,
        bd_b=bd_b, fc_w=fc_w, fc_b=fc_b, fm_w=fm_w, fm_b=fm_b, cv_w=cv_w,
        cv_b=cv_b).items()}
    wc8 = np.zeros((128, W8TOT), F8)
    wc8[:, 0:576] = _conv_w8(g["fc_w"])
    wc8[0:40, C_FUSE:C_FUSE + 768] = _fuse_w8(g["fuse_w"])
    wc8[:, C_LBMS:C_LBMS + 2048] = _lbms_w8(g["fm_w"])
    wcb = _wcb(g["cv_w"], g["cv_b"], g["se_w1"], g["se_w2"], g["bd_w"])
    fcol = _fcol(g["fc_b"], g["cv_w"], g["cv_b"], g["fm_b"], g["bd_b"],
                 g["fuse_b"])
    xb = np.asarray(x, np.float32)
    yb = np.asarray(y, np.float32)
    xp, yhp, ycp = _pack_inputs(xb, yb)
    in_maps = [
        {"xp": xp[i], "yhp": yhp[i], "ycp": ycp[i],
         "wc8": wc8, "wcb": wcb, "fcol": fcol}
        for i in range(xb.shape[0])
    ]
    return nc, in_maps


def kernel(x, y, fuse_w, fuse_b, se_w1, se_w2, bd_w, bd_b,
           fc_w, fc_b, fm_w, fm_b, cv_w, cv_b):
    global LAST_RESULT
    from concourse.bass_utils import run_bass_kernel_spmd

    nc, in_maps = prepare(x, y, fuse_w, fuse_b, se_w1, se_w2, bd_w, bd_b,
                          fc_w, fc_b, fm_w, fm_b, cv_w, cv_b)
    res = run_bass_kernel_spmd(nc, in_maps, core_ids=list(range(8)))
    LAST_RESULT = res
    outs = []
    for i in range(len(in_maps)):
        ot = np.asarray(res.results[i]["out"], np.float32)   # [NT, 96, W]
        full = ot.reshape(NT, SB, 16, W).transpose(2, 0, 1, 3) \
                 .reshape(16, NT * SB, W)[:, :H, :]
        outs.append(full)
    return np.stack(outs)


# revision 9
# speedup vs baseline: 1.0542x; 1.0066x over previous
"""Trainium2 Bass kernel for nn_Boundary_Enchance (dense_cnn), v2.

Pure data parallel: core i processes batch image i.  Compute is fp8-e4m3
DoubleRow on the PE for the fuse 1x1, the 3x3 conv, and the merged
mask+boundary head (validated rel-err 0.004 on host); the final 1->16
expansion stays bf16 (fp8 weights there cost ~2% systematic error).

Layout tricks:
  - XF tiles [128, 1028] fp8: cols 0..513 = x (8 rows x 16ch, image cols
    -1..512, host-packed), cols 514..1027 = F = relu(fuse(y)) written by the
    fuse evacuation.  The 3x3 conv is THREE DoubleRow matmuls (one per dx
    tap): the DR pair (stride 514) contracts x and F simultaneously, K_eff
    = 256 = 8 rows x 32 ch.  dy taps ride the row-Toeplitz lhsT; dx taps
    are rhs base-column shifts.
  - fuse: one DR matmul per 8-row tile, pair (j, j+2) with zero second
    weights; bias folded into the evacuation tensor_scalar (per-tile bias
    column variants handle the image edges).
  - mask+boundary (LBM): per strip one DR matmul with M=128 accumulating 8
    strips into ONE psum bank (strip g -> mask logits at partitions 6g+i,
    boundary at 64+6g+i, zeros elsewhere).  Tail nonlinearities run ONCE
    per 8 strips: sigmoid (with per-partition bias = head biases), add,
    min -> sv group tile [48, 512].
  - final expansion per strip: either an LC bf16 matmul + psum evacuation,
    or a replicating SBUF DMA + per-partition affine (x cv_w + cv_b) on
    the DVE at 4x rate ("REP path") -- mixed to balance PE/DMA/engines.
"""

import numpy as np
import ml_dtypes

BF16 = ml_dtypes.bfloat16
F8 = ml_dtypes.float8_e4m3

H = 512
W = 512
SB = 6
NT = (H + SB - 1) // SB          # 86 strips
XW = 514                          # x / F block width (image cols -1..512)
PW = 2 * XW                       # XF tile width
FW = 514                          # fcc slot width (512 + 2 pad)
NG = (NT + 7) // 8                # strip groups of 8

RING_YH = 16
TAIL_LAG = 8                      # tails lag fronts in phase B
N_WARM = 16
CHAIN_POOL = False


REP_FRAC = 3                      # u % REP_FRAC == 1 -> REP; 0 disables
CONV_DEPTH = 2                    # strips per conv psum tile group
CHAIN_LAG = 3
FIN_LAG = 8
ACT_W = 0.97
BAL_DECAY = 1.0
FUSE_BUFS = 2
LC_BUFS = 2
AFF_MOD = 1000000


REP_END = 72                      # strips >= REP_END never use REP


REP_MID = 0                       # strips in [REP_MID, REP_END) use u%2==1
OG_SPLIT = False


def _is_rep(u):
    if u >= REP_END:
        return False
    if REP_MID and u >= REP_MID:
        return (u % 2) == 1
    return REP_FRAC == 1 or (REP_FRAC > 0 and (u % REP_FRAC) == 1)

# wc8 fp8 column map
C_CONV = 0                        # 3 taps x [2,96] = 576
C_FUSE = 576                      # 3 variants x [2,128] = 768
C_LBMS = 576 + 768                # 8 variants x [2,128] = 2048
W8TOT = C_LBMS + 2048
# wcb bf16 column map
B_LCW = 0                         # 8 variants x 96 = 768
B_SEL = 768                       # 96
B_W1 = 768 + 96                   # 96
B_W2 = 768 + 192                  # 128
B_PSBW = 768 + 320                # 2048 (partitions 96..125)
B_ONES = B_PSBW + 2048            # 512 cols of 1.0 at partition 48
WBTOT = B_ONES + 512
# fcol f32 columns: 0 fcb, 1 cvw, 2 cvb, 3 sigbias, 4..6 fuse bias variants
FCOLS = 7

_cache = {}


# ---------------------------------------------------------------------------
# host-side packing
# ---------------------------------------------------------------------------

def _conv_w8(fc_w):
    """[128, 576] fp8: tap d at cols 192d..192d+191, pair p block of 96:
    w[k=r*16+c, 192d+96p+i*16+oc] = fc_w[oc, 16p+c, r-i, d] for r-i in 0..2."""
    out = np.zeros((128, 576), np.float32)
    for d in range(3):
        for p in range(2):
            for i in range(SB):
                for ky in range(3):
                    r = i + ky
                    out[r * 16:r * 16 + 16,
                        192 * d + 96 * p + i * 16:192 * d + 96 * p + i * 16 + 16] = \
                        fc_w[:, 16 * p:16 * p + 16, ky, d].T
    return out.astype(F8)


def _fuse_w8(fuse_w):
    """[40, 768] fp8: 3 identical variants (edges handled by evac bias cols):
    w[k=r*5+yc, 256v+0*128+r*16+oc] = fuse_w[oc, yc]; pair1 block zero."""
    out = np.zeros((40, 768), np.float32)
    for v in range(3):
        for r in range(8):
            out[r * 5:r * 5 + 5, 256 * v + r * 16:256 * v + r * 16 + 16] = \
                fuse_w[:, :, 0, 0].T
    return out.astype(F8)


def _lbms_w8(fm_w):
    """[128, 2048] fp8 static part: variant g at cols 256g..: pair0
    [k=i*16+c, m=6g+i] = fm diff; rows 96..125 (bd) filled on device."""
    out = np.zeros((128, 2048), np.float32)
    d = fm_w[1, :, 0, 0] - fm_w[0, :, 0, 0]
    for g in range(8):
        for i in range(SB):
            out[i * 16:i * 16 + 16, 256 * g + 6 * g + i] = d
    return out.astype(F8)


def _wcb(cv_w, cv_b, se_w1, se_w2, bd_w):
    out = np.zeros((128, WBTOT), np.float32)
    # LCW: 8 variants [49 rows, 96]: rows 6g+i -> cv_w; row 48 (ones) -> cv_b
    for g in range(8):
        for i in range(SB):
            out[6 * g + i, B_LCW + 96 * g + i * 16:B_LCW + 96 * g + i * 16 + 16] = \
                cv_w[:, 0, 0, 0]
        for i in range(SB):
            out[48, B_LCW + 96 * g + i * 16:B_LCW + 96 * g + i * 16 + 16] = cv_b
    out[48, B_ONES:B_ONES + 512] = 1.0
    # SEL: [r*16+oc, oc] = 1/NPIX for r in 1..6
    for r in range(1, 7):
        for fc in range(16):
            out[r * 16 + fc, B_SEL + fc] = 1.0 / float(H * W)
    # W1
    out[0:16, B_W1:B_W1 + 16] = se_w1.T
    # W2: cols 96 + r*5 + yc <- se_w2.T (targets se_bc partitions 96..125)
    for r in range(SB):
        out[0:16, B_W2 + 96 + r * 5:B_W2 + 96 + r * 5 + 5] = se_w2.T
    # PSBW: partitions 96..125: [96 + r*5+yc, 256g + 64+6g+r] = bd diff
    dbd = bd_w[1, :, 0, 0] - bd_w[0, :, 0, 0]
    for g in range(8):
        for r in range(SB):
            out[96 + r * 5:96 + r * 5 + 5,
                B_PSBW + 256 * g + 64 + 6 * g + r] = dbd
    return out.astype(BF16)


def _fcol(fc_b, cv_w, cv_b, fm_b, bd_b, fuse_b):
    out = np.zeros((128, FCOLS), np.float32)
    for i in range(SB):
        out[i * 16:(i + 1) * 16, 0] = fc_b          # conv bias (+relu evac)
        out[i * 16:(i + 1) * 16, 1] = cv_w[:, 0, 0, 0]
        out[i * 16:(i + 1) * 16, 2] = cv_b
    for g in range(8):
        out[6 * g:6 * g + 6, 3] = fm_b[1] - fm_b[0]      # sigmoid bias: mask
        out[64 + 6 * g:64 + 6 * g + 6, 3] = bd_b[1] - bd_b[0]  # boundary
    for r in range(8):
        out[r * 16:r * 16 + 16, 4] = fuse_b          # interior fuse bias
        out[r * 16:r * 16 + 16, 5] = fuse_b if r > 0 else 0.0   # first tile
        out[r * 16:r * 16 + 16, 6] = fuse_b if r < 3 else 0.0   # last tile
    return out


def _pack_inputs(xb, yb):
    """xp [NT,128,514] fp8; yhp [NT,40,512] fp8; ycp [NT,30,512] fp8."""
    B = xb.shape[0]
    x8 = xb.astype(F8)
    y8 = yb.astype(F8)
    xpad = np.zeros((B, 16, SB * NT + 8, W + 2), F8)
    xpad[:, :, 1:H + 1, 1:W + 1] = x8
    ridx = SB * np.arange(NT)[:, None] + np.arange(8)[None, :]
    xp = xpad[:, :, ridx, :].transpose(0, 2, 3, 1, 4).reshape(B, NT, 128, XW)
    ypad = np.zeros((B, 5, SB * NT + 8, W), F8)
    ypad[:, :, 1:H + 1, :] = y8
    yhp = ypad[:, :, ridx, :].transpose(0, 2, 3, 1, 4).reshape(B, NT, 40, W)
    cidx = SB * np.arange(NT)[:, None] + 1 + np.arange(SB)[None, :]
    ycp = ypad[:, :, cidx, :].transpose(0, 2, 3, 1, 4).reshape(B, NT, 30, W)
    return (np.ascontiguousarray(xp), np.ascontiguousarray(yhp),
            np.ascontiguousarray(ycp))


# ---------------------------------------------------------------------------
# bass graph
# ---------------------------------------------------------------------------

def _pairs(v, pair_stride, n, base=0):
    """3-dim DR rhs AP [[part],[pair_stride,2],[1,n]] from a 2-dim view."""
    a = v.unsqueeze(1).copy()
    a.ap[1] = [pair_stride, 2]
    a.ap[2] = [1, n]
    return a


def _build():
    import concourse.bacc as bacc
    import concourse.tile as tile
    from concourse import mybir

    f32 = mybir.dt.float32
    bf16 = mybir.dt.bfloat16
    f8 = mybir.dt.float8e4
    AF = mybir.ActivationFunctionType
    ALU = mybir.AluOpType
    DR = mybir.MatmulPerfMode.DoubleRow

    def _pick(act_cost, dve_cost):
        bal = _cache.setdefault("_bal", [0.0, 0.0])
        bal[0] *= BAL_DECAY
        bal[1] *= BAL_DECAY
        if (bal[0] + act_cost) * ACT_W <= bal[1] + dve_cost:
            bal[0] += act_cost
            return 0
        bal[1] += dve_cost
        return 1

    nc = bacc.Bacc("TRN2", target_bir_lowering=False)
    xp_ext = nc.declare_dram_parameter("xp", [NT, 128, XW], f8, isOutput=False)
    yhp_ext = nc.declare_dram_parameter("yhp", [NT, 40, W], f8, isOutput=False)
    ycp_ext = nc.declare_dram_parameter("ycp", [NT, 30, W], f8, isOutput=False)
    wc8_ext = nc.declare_dram_parameter("wc8", [128, W8TOT], f8, isOutput=False)
    wcb_ext = nc.declare_dram_parameter("wcb", [128, WBTOT], bf16,
                                        isOutput=False)
    fcol_ext = nc.declare_dram_parameter("fcol", [128, FCOLS], f32,
                                         isOutput=False)
    out_ext = nc.declare_dram_parameter("out", [NT, 96, W], bf16,
                                        isOutput=True)

    with tile.TileContext(nc) as tc:
        with (
            tc.tile_pool(name="singles", bufs=1) as singles,
            tc.tile_pool(name="ps_fuse", bufs=FUSE_BUFS, space="PSUM")
                as ps_fuse,
            tc.tile_pool(name="ps_conv", bufs=2, space="PSUM") as ps_conv,
            tc.tile_pool(name="ps_lc", bufs=LC_BUFS, space="PSUM") as ps_lc,
        ):
            # ---------------- static tiles + warmup -----------------------
            wtile = singles.tile([128, 256], bf16, tag="wtile")
            nc.vector.memset(wtile[:, :], 0.0)
            wps = ps_conv.tile([96, 1024], f32, tag="conv", name="warmps")
            for i in range(N_WARM):
                nc.tensor.matmul(wps[0:96, 0:256], lhsT=wtile[:, 0:96],
                                 rhs=wtile[:, :], start=(i == 0),
                                 stop=(i == N_WARM - 1))

            wc8 = singles.tile([128, W8TOT], f8, tag="wc8")
            nc.sync.dma_start(out=wc8[:, :], in_=wc8_ext[:, :])
            wcb = singles.tile([128, WBTOT], bf16, tag="wcb")
            fcol = singles.tile([128, FCOLS], f32, tag="fcol")
            nc.sync.dma_start(out=fcol[:, :], in_=fcol_ext[:, :])

            def conv_lhsT(d):
                a = wc8[:, 192 * d:192 * (d + 1)].unsqueeze(1).copy()
                a.ap[1] = [96, 2]
                a.ap[2] = [1, 96]
                return a

            def fuse_lhsT(v):
                a = wc8[0:40, C_FUSE + 256 * v:C_FUSE + 256 * (v + 1)] \
                    .unsqueeze(1).copy()
                a.ap[1] = [128, 2]
                a.ap[2] = [1, 128]
                return a

            def lbms_lhsT(g):
                a = wc8[0:126, C_LBMS + 256 * g:C_LBMS + 256 * (g + 1)] \
                    .unsqueeze(1).copy()
                a.ap[1] = [128, 2]
                a.ap[2] = [1, 128]
                return a

            SELW = wcb[:, B_SEL:B_SEL + 96]
            W1L = wcb[:, B_W1:B_W1 + 96]
            W2R = wcb[:, B_W2:B_W2 + 128]

            # ---------------- data tiles ----------------------------------
            xf = singles.tile([128, NT * PW], f8, tag="xf")
            yh = singles.tile([40, RING_YH * XW], f8, tag="yh")
            fcc = singles.tile([128, NT * FW], f8, tag="fcc")
            og = [singles.tile([96, 8 * W], bf16, tag=f"og{k}", name=f"og{k}")
                  for k in (0, 1, 2)]
            sg = [singles.tile([112, W], bf16, tag=f"sg{k}", name=f"sg{k}")
                  for k in (0, 1)]
            sgB = [singles.tile([48, W], bf16, tag=f"sgB{k}", name=f"sgB{k}")
                   for k in (0, 1)]
            svg = [singles.tile([49, W], bf16, tag=f"svg{k}", name=f"svg{k}")
                   for k in (0, 1, 2)]
            rep = [singles.tile([96, W], bf16, tag=f"rep{k}", name=f"rep{k}")
                   for k in (0, 1, 2, 3)]
            Ra = singles.tile([128, NT], f32, tag="Ra")
            nc.vector.memset(Ra[:, :], 0.0)
            Rb = singles.tile([128, NT], f32, tag="Rb")
            nc.vector.memset(Rb[:, :], 0.0)

            # presets: XF F-block pad cols (t=0,513 per ring slot); yh pad
            # cols 512/513 per slot; fcc pad cols 512/513 per slot.
            m = xf[:, XW:XW + 1].unsqueeze(1).copy()
            m.ap[1] = [PW, NT]
            m.ap[2] = [XW - 1, 2]
            nc.gpsimd.memset(m, 0.0)
            m = yh[:, W:W + 1].unsqueeze(1).copy()
            m.ap[1] = [XW, RING_YH]
            m.ap[2] = [1, 2]
            nc.gpsimd.memset(m, 0.0)
            m = fcc[:, W:W + 1].unsqueeze(1).copy()
            m.ap[1] = [FW, NT]
            m.ap[2] = [1, 2]
            nc.gpsimd.memset(m, 0.0)

            # ---------------- phase A + fronts ----------------------------
            def xslot(t):
                return xf[:, t * PW:t * PW + PW]

            def issue_fuse(t):
                if t % 8 == 0:
                    n = min(8, NT - t)
                    s1 = (t % RING_YH)
                    nc.sync.dma_start(
                        out=yh[:, s1 * XW:(s1 + n) * XW].rearrange(
                            "p (s c) -> p s c", s=n)[:, :, 0:W],
                        in_=yhp_ext[t:t + n, :, :].rearrange("s p j -> p s j"))
                    nc.sync.dma_start(
                        out=xf[:, t * PW:(t + n) * PW].rearrange(
                            "p (s c) -> p s c", s=n)[:, :, 0:XW],
                        in_=xp_ext[t:t + n, :, :].rearrange("s p j -> p s j"))
                yv = yh[0:40, (t % RING_YH) * XW:(t % RING_YH) * XW + W]
                fps = ps_fuse.tile([128, 512], f32, tag="fuse")
                v = 1 if t == 0 else (2 if t == NT - 1 else 0)
                nc.tensor.matmul(fps[:, :], lhsT=fuse_lhsT(v),
                                 rhs=_pairs(yv, 2, W), start=True, stop=True,
                                 perf_mode=DR)
                fdst = xslot(t)[:, XW + 1:XW + 1 + W]
                bv = 4 if 0 < t < NT - 1 else (5 if t == 0 else 6)
                bias = fcol[:, bv:bv + 1]
                if _pick(612, 658) == 0:
                    nc.scalar.activation(out=fdst, in_=fps[:, :], func=AF.Relu,
                                         bias=bias, accum_out=Ra[:, t:t + 1])
                else:
                    nc.vector.tensor_scalar(out=fdst, in0=fps[:, :],
                                            scalar1=bias, scalar2=0.0,
                                            op0=ALU.add, op1=ALU.max,
                                            accum_out=Rb[:, t:t + 1])

            cpair = {}

            def issue_front(s):
                xv = xslot(s)
                CD = CONV_DEPTH
                if s % CD == 0:
                    cpair[0] = ps_conv.tile([96, CD * 512], f32, tag="conv",
                                            name="cps")
                cps = cpair[0]
                half = (s % CD) * 512
                for d in range(3):
                    nc.tensor.matmul(cps[:, half:half + 512],
                                     lhsT=conv_lhsT(d),
                                     rhs=_pairs(xv[:, d:], XW, W),
                                     start=(d == 0), stop=(d == 2),
                                     perf_mode=DR)
                nb = s % CD + 1
                if s % CD != CD - 1 and s != NT - 1:
                    return
                s0 = s - nb + 1
                dst = fcc[0:96, s0 * FW:s0 * FW + 1].unsqueeze(1).copy()
                dst.ap[1] = [FW, nb]
                dst.ap[2] = [1, W]
                src = cps[:, 0:nb * 512]
                if _pick(185 + nb * 427, 125 + nb * 533) == 0:
                    nc.scalar.activation(out=dst, in_=src, func=AF.Relu,
                                         bias=fcol[0:96, 0:1])
                else:
                    nc.vector.tensor_scalar(out=dst, in0=src,
                                            scalar1=fcol[0:96, 0:1],
                                            scalar2=0.0,
                                            op0=ALU.add, op1=ALU.max)

            for t in range(NT):
                if t == 2:
                    nc.sync.dma_start(out=wcb[:, :], in_=wcb_ext[:, :])
                    for k in (0, 1, 2):
                        nc.sync.dma_start(
                            out=svg[k][48:49, :],
                            in_=wcb[48:49, B_ONES:B_ONES + 512])
                issue_fuse(t)

            # ---------------- SE chain ------------------------------------
            R_bf = singles.tile([128, NT], bf16, tag="Rbf")
            nc.vector.tensor_add(out=R_bf[:, :], in0=Ra[:, :], in1=Rb[:, :])
            gps = ps_fuse.tile([96, NT], f32, tag="fuse")
            nc.tensor.matmul(gps[:, :], lhsT=SELW, rhs=R_bf[:, :],
                             start=True, stop=True)
            gap_f = singles.tile([96, 1], f32, tag="gapf")
            nc.vector.reduce_sum(out=gap_f[:, :], in_=gps[:, :],
                                 axis=mybir.AxisListType.X)
            gap_bf = singles.tile([128, 1], bf16, tag="gap")
            nc.vector.memset(gap_bf[:, :], 0.0)
            nc.vector.tensor_copy(out=gap_bf[0:96, :], in_=gap_f[:, :])
            hps = ps_fuse.tile([96, 1], f32, tag="fuse")
            nc.tensor.matmul(hps[:, :], lhsT=W1L, rhs=gap_bf[:, :],
                             start=True, stop=True)
            h_bf = singles.tile([128, 1], bf16, tag="hbf")
            nc.vector.memset(h_bf[:, :], 0.0)
            nc.scalar.activation(out=h_bf[0:96, :], in_=hps[:, :], func=AF.Relu)
            sps = ps_fuse.tile([128, 1], f32, tag="fuse")
            nc.tensor.matmul(sps[:, :], lhsT=W2R, rhs=h_bf[:, :],
                             start=True, stop=True)
            se_bc = singles.tile([128, 1], f32, tag="sebc")
            nc.scalar.activation(out=se_bc[:, :], in_=sps[:, :],
                                 func=AF.Sigmoid)
            # fill LBMS bd rows (96..125): wc8 <- PSBW * se
            nc.scalar.activation(out=wc8[96:126, C_LBMS:C_LBMS + 1024],
                                 in_=wcb[96:126, B_PSBW:B_PSBW + 1024],
                                 func=AF.Copy, scale=se_bc[96:126, :])
            nc.vector.tensor_scalar(
                out=wc8[96:126, C_LBMS + 1024:C_LBMS + 2048],
                in0=wcb[96:126, B_PSBW + 1024:B_PSBW + 2048],
                scalar1=se_bc[96:126, :], scalar2=0.0,
                op0=ALU.mult, op1=ALU.add)

            # ---------------- tails ---------------------------------------
            def issue_tail(u):
                g = u % 8
                G = u // 8
                gp = _cache.setdefault("_gp", {})
                if g == 0:
                    gp[G] = ps_fuse.tile([128, 512], f32, tag="fuse",
                                         name="grp")
                GP = gp[G]
                fv = fcc[0:126, u * FW:u * FW + W]
                nc.tensor.matmul(GP[:, :], lhsT=lbms_lhsT(g),
                                 rhs=_pairs(fv, 2, W), start=(g == 0),
                                 stop=(g == 7 or u == NT - 1), perf_mode=DR)
            def issue_chain(G):
                u = min(8 * G + 7, NT - 1)
                ng = u - 8 * G + 1           # strips in this group
                gp = _cache.setdefault("_gp", {})
                GP = gp[G]
                sgt, svt = sg[G % 2], svg[G % 3]
                np_ = 64 + 6 * ng
                bal = _cache.setdefault("_bal", [0.0, 0.0])
                bal[0] += 612.0
                nc.scalar.activation(out=sgt[0:np_, :], in_=GP[0:np_, :],
                                     func=AF.Sigmoid, bias=fcol[0:np_, 3:4])
                sbt = sgB[G % 2]
                ce = nc.gpsimd if CHAIN_POOL else nc.vector
                if not CHAIN_POOL:
                    bal[1] += 713.0
                ce.tensor_copy(out=sbt[0:6 * ng, :],
                               in_=sgt[64:64 + 6 * ng, :])
                ce.tensor_add(out=svt[0:6 * ng, :],
                              in0=sgt[0:6 * ng, :],
                              in1=sbt[0:6 * ng, :])
                ce.tensor_scalar(out=svt[0:6 * ng, :],
                                 in0=svt[0:6 * ng, :],
                                 scalar1=1.0, scalar2=0.0,
                                 op0=ALU.min, op1=ALU.add)

            def og_dma(G, half=None):
                ng = min(8, NT - 8 * G)
                lo, hi = 0, ng
                if half == 0:
                    hi = min(4, ng)
                elif half == 1:
                    lo = 4
                    if ng <= 4:
                        return
                nc.sync.dma_start(
                    out=out_ext[8 * G + lo:8 * G + hi, :, :].rearrange(
                        "s p j -> p s j"),
                    in_=og[G % 3][0:96, lo * W:hi * W])

            def issue_final(uu):
                G = uu // 8
                gg = uu % 8
                svt = svg[G % 3]
                if True:
                    dst = og[G % 3][:, gg * W:gg * W + W]
                    if _is_rep(uu):
                        rt = rep[_cache.setdefault('_rr', [0])[0] % 4]
                        _cache['_rr'][0] += 1
                        src = svt[6 * gg:6 * gg + 6, :].unsqueeze(1).copy()
                        src.ap[1] = [0, 16]
                        nc.sync.dma_start(out=rt[:, :], in_=src)
                        k = _cache.setdefault("_repn", [0])
                        k[0] += 1
                        eng = nc.gpsimd if k[0] % AFF_MOD != 0 else nc.vector
                        eng.tensor_scalar(out=dst, in0=rt[:, :],
                                          scalar1=fcol[0:96, 1:2],
                                          scalar2=fcol[0:96, 2:3],
                                          op0=ALU.mult, op1=ALU.add)
                    else:
                        ops = ps_lc.tile([96, 512], f32, tag="lc")
                        nc.tensor.matmul(ops[:, :],
                                         lhsT=wcb[0:49, B_LCW + 96 * gg:
                                                  B_LCW + 96 * (gg + 1)],
                                         rhs=svt[0:49, :],
                                         start=True, stop=True)
                        if _pick(612, 658) == 0:
                            nc.scalar.activation(out=dst, in_=ops[:, :],
                                                 func=AF.Copy)
                        else:
                            nc.vector.tensor_copy(out=dst, in_=ops[:, :])

            for s0 in range(0, NT, 8):
                n = min(8, NT - s0)
                nc.sync.dma_start(
                    out=fcc[96:126, s0 * FW:(s0 + n) * FW].rearrange(
                        "p (s c) -> p s c", s=n)[:, :, 0:W],
                    in_=ycp_ext[s0:s0 + n, :, :].rearrange("s p j -> p s j"))
            tc = [0]
            fc_ = [0]
            cc = [0]
            vu = [-1]
            s = 0
            while fc_[0] < NT:
                if s < NT:
                    issue_front(s)
                pace = 1 if s < NT else 2
                if s >= TAIL_LAG:
                    vu[0] += pace
                for _ in range(pace):
                    if tc[0] < NT and tc[0] <= vu[0]:
                        issue_tail(tc[0])
                        tc[0] += 1
                while cc[0] < NG and vu[0] - min(8 * cc[0] + 7, NT - 1) \
                        >= CHAIN_LAG:
                    issue_chain(cc[0])
                    cc[0] += 1
                uf_max = vu[0] - CHAIN_LAG - FIN_LAG
                nfin = pace
                while fc_[0] <= uf_max and fc_[0] < NT and nfin > 0 \
                        and fc_[0] // 8 < cc[0]:
                    uf = fc_[0]
                    fc_[0] += 1
                    nfin -= 1
                    issue_final(uf)
                    if uf % 8 == 7 or uf == NT - 1:
                        og_dma(uf // 8)
                s += 1
    nc.compile()
    for k in ("_gp", "_lcn", "_repn", "_bal", "_rr", "_rq", "_fc", "_cc"):
        _cache.pop(k, None)
    return nc


# ---------------------------------------------------------------------------
# entry point
# ---------------------------------------------------------------------------

LAST_RESULT = None


def prepare(x, y, fuse_w, fuse_b, se_w1, se_w2, bd_w, bd_b,
            fc_w, fc_b, fm_w, fm_b, cv_w, cv_b):
    if "nc" not in _cache:
        _cache["nc"] = _build()
    nc = _cache["nc"]
    g = {k: np.asarray(v, np.float32) for k, v in dict(
        fuse_w=fuse_w, fuse_b=fuse_b, se_w1=se_w1, se_w2=se_w2, bd_w=bd_w,
        bd_b=bd_b, fc_w=fc_w, fc_b=fc_b, fm_w=fm_w, fm_b=fm_b, cv_w=cv_w,
        cv_b=cv_b).items()}
    wc8 = np.zeros((128, W8TOT), F8)
    wc8[:, 0:576] = _conv_w8(g["fc_w"])
    wc8[0:40, C_FUSE:C_FUSE + 768] = _fuse_w8(g["fuse_w"])
    wc8[:, C_LBMS:C_LBMS + 2048] = _lbms_w8(g["fm_w"])
    wcb = _wcb(g["cv_w"], g["cv_b"], g["se_w1"], g["se_w2"], g["bd_w"])
    fcol = _fcol(g["fc_b"], g["cv_w"], g["cv_b"], g["fm_b"], g["bd_b"],
                 g["fuse_b"])
    xb = np.asarray(x, np.float32)
    yb = np.asarray(y, np.float32)
    xp, yhp, ycp = _pack_inputs(xb, yb)
    in_maps = [
        {"xp": xp[i], "yhp": yhp[i], "ycp": ycp[i],
         "wc8": wc8, "wcb": wcb, "fcol": fcol}
        for i in range(xb.shape[0])
    ]
    return nc, in_maps


def kernel(x, y, fuse_w, fuse_b, se_w1, se_w2, bd_w, bd_b,
           fc_w, fc_b, fm_w, fm_b, cv_w, cv_b):
    global LAST_RESULT
    from concourse.bass_utils import run_bass_kernel_spmd

    nc, in_maps = prepare(x, y, fuse_w, fuse_b, se_w1, se_w2, bd_w, bd_b,
                          fc_w, fc_b, fm_w, fm_b, cv_w, cv_b)
    res = run_bass_kernel_spmd(nc, in_maps, core_ids=list(range(8)))
    LAST_RESULT = res
    outs = []
    for i in range(len(in_maps)):
        ot = np.asarray(res.results[i]["out"], np.float32)   # [NT, 96, W]
        full = ot.reshape(NT, SB, 16, W).transpose(2, 0, 1, 3) \
                 .reshape(16, NT * SB, W)[:, :H, :]
        outs.append(full)
    return np.stack(outs)


# revision 10
# speedup vs baseline: 1.0641x; 1.0094x over previous
"""Trainium2 Bass kernel for nn_Boundary_Enchance (dense_cnn), v2.

Pure data parallel: core i processes batch image i.  Compute is fp8-e4m3
DoubleRow on the PE for the fuse 1x1, the 3x3 conv, and the merged
mask+boundary head (validated rel-err 0.004 on host); the final 1->16
expansion stays bf16 (fp8 weights there cost ~2% systematic error).

Layout tricks:
  - XF tiles [128, 1028] fp8: cols 0..513 = x (8 rows x 16ch, image cols
    -1..512, host-packed), cols 514..1027 = F = relu(fuse(y)) written by the
    fuse evacuation.  The 3x3 conv is THREE DoubleRow matmuls (one per dx
    tap): the DR pair (stride 514) contracts x and F simultaneously, K_eff
    = 256 = 8 rows x 32 ch.  dy taps ride the row-Toeplitz lhsT; dx taps
    are rhs base-column shifts.
  - fuse: one DR matmul per 8-row tile, pair (j, j+2) with zero second
    weights; bias folded into the evacuation tensor_scalar (per-tile bias
    column variants handle the image edges).
  - mask+boundary (LBM): per strip one DR matmul with M=128 accumulating 8
    strips into ONE psum bank (strip g -> mask logits at partitions 6g+i,
    boundary at 64+6g+i, zeros elsewhere).  Tail nonlinearities run ONCE
    per 8 strips: sigmoid (with per-partition bias = head biases), add,
    min -> sv group tile [48, 512].
  - final expansion per strip: either an LC bf16 matmul + psum evacuation,
    or a replicating SBUF DMA + per-partition affine (x cv_w + cv_b) on
    the DVE at 4x rate ("REP path") -- mixed to balance PE/DMA/engines.
"""

import numpy as np
import ml_dtypes

BF16 = ml_dtypes.bfloat16
F8 = ml_dtypes.float8_e4m3

H = 512
W = 512
SB = 6
NT = (H + SB - 1) // SB          # 86 strips
XW = 514                          # x / F block width (image cols -1..512)
PW = 2 * XW                       # XF tile width
FW = 514                          # fcc slot width (512 + 2 pad)
NG = (NT + 7) // 8                # strip groups of 8

RING_YH = 16
TAIL_LAG = 8                      # tails lag fronts in phase B
N_WARM = 16
CHAIN_POOL = False


REP_FRAC = 3                      # u % REP_FRAC == 1 -> REP; 0 disables
CONV_DEPTH = 2                    # strips per conv psum tile group
CHAIN_LAG = 3
FIN_LAG = 8
ACT_W = 0.97
BAL_DECAY = 1.0
FUSE_BUFS = 2
LC_BUFS = 2
AFF_MOD = 1000000


REP_END = 72                      # strips >= REP_END never use REP


REP_PH = 1
REP_MID = 0                       # strips in [REP_MID, REP_END) use u%2==1
OG_SPLIT = False


def _is_rep(u):
    if u >= REP_END:
        return False
    if REP_MID and u >= REP_MID:
        return (u % 2) == 1
    return REP_FRAC == 1 or (REP_FRAC > 0 and (u % REP_FRAC) == REP_PH)

# wc8 fp8 column map
C_CONV = 0                        # 3 taps x [2,96] = 576
C_FUSE = 576                      # 3 variants x [2,128] = 768
C_LBMS = 576 + 768                # 8 variants x [2,128] = 2048
W8TOT = C_LBMS + 2048
# wcb bf16 column map
B_LCW = 0                         # 8 variants x 96 = 768
B_SEL = 768                       # 96
B_W1 = 768 + 96                   # 96
B_W2 = 768 + 192                  # 128
B_PSBW = 768 + 320                # 2048 (partitions 96..125)
B_ONES = B_PSBW + 2048            # 512 cols of 1.0 at partition 48
WBTOT = B_ONES + 512
# fcol f32 columns: 0 fcb, 1 cvw, 2 cvb, 3 sigbias, 4..6 fuse bias variants
FCOLS = 7

_cache = {}


# ---------------------------------------------------------------------------
# host-side packing
# ---------------------------------------------------------------------------

def _conv_w8(fc_w):
    """[128, 576] fp8: tap d at cols 192d..192d+191, pair p block of 96:
    w[k=r*16+c, 192d+96p+i*16+oc] = fc_w[oc, 16p+c, r-i, d] for r-i in 0..2."""
    out = np.zeros((128, 576), np.float32)
    for d in range(3):
        for p in range(2):
            for i in range(SB):
                for ky in range(3):
                    r = i + ky
                    out[r * 16:r * 16 + 16,
                        192 * d + 96 * p + i * 16:192 * d + 96 * p + i * 16 + 16] = \
                        fc_w[:, 16 * p:16 * p + 16, ky, d].T
    return out.astype(F8)


def _fuse_w8(fuse_w):
    """[40, 768] fp8: 3 identical variants (edges handled by evac bias cols):
    w[k=r*5+yc, 256v+0*128+r*16+oc] = fuse_w[oc, yc]; pair1 block zero."""
    out = np.zeros((40, 768), np.float32)
    for v in range(3):
        for r in range(8):
            out[r * 5:r * 5 + 5, 256 * v + r * 16:256 * v + r * 16 + 16] = \
                fuse_w[:, :, 0, 0].T
    return out.astype(F8)


def _lbms_w8(fm_w):
    """[128, 2048] fp8 static part: variant g at cols 256g..: pair0
    [k=i*16+c, m=6g+i] = fm diff; rows 96..125 (bd) filled on device."""
    out = np.zeros((128, 2048), np.float32)
    d = fm_w[1, :, 0, 0] - fm_w[0, :, 0, 0]
    for g in range(8):
        for i in range(SB):
            out[i * 16:i * 16 + 16, 256 * g + 6 * g + i] = d
    return out.astype(F8)


def _wcb(cv_w, cv_b, se_w1, se_w2, bd_w):
    out = np.zeros((128, WBTOT), np.float32)
    # LCW: 8 variants [49 rows, 96]: rows 6g+i -> cv_w; row 48 (ones) -> cv_b
    for g in range(8):
        for i in range(SB):
            out[6 * g + i, B_LCW + 96 * g + i * 16:B_LCW + 96 * g + i * 16 + 16] = \
                cv_w[:, 0, 0, 0]
        for i in range(SB):
            out[48, B_LCW + 96 * g + i * 16:B_LCW + 96 * g + i * 16 + 16] = cv_b
    out[48, B_ONES:B_ONES + 512] = 1.0
    # SEL: [r*16+oc, oc] = 1/NPIX for r in 1..6
    for r in range(1, 7):
        for fc in range(16):
            out[r * 16 + fc, B_SEL + fc] = 1.0 / float(H * W)
    # W1
    out[0:16, B_W1:B_W1 + 16] = se_w1.T
    # W2: cols 96 + r*5 + yc <- se_w2.T (targets se_bc partitions 96..125)
    for r in range(SB):
        out[0:16, B_W2 + 96 + r * 5:B_W2 + 96 + r * 5 + 5] = se_w2.T
    # PSBW: partitions 96..125: [96 + r*5+yc, 256g + 64+6g+r] = bd diff
    dbd = bd_w[1, :, 0, 0] - bd_w[0, :, 0, 0]
    for g in range(8):
        for r in range(SB):
            out[96 + r * 5:96 + r * 5 + 5,
                B_PSBW + 256 * g + 64 + 6 * g + r] = dbd
    return out.astype(BF16)


def _fcol(fc_b, cv_w, cv_b, fm_b, bd_b, fuse_b):
    out = np.zeros((128, FCOLS), np.float32)
    for i in range(SB):
        out[i * 16:(i + 1) * 16, 0] = fc_b          # conv bias (+relu evac)
        out[i * 16:(i + 1) * 16, 1] = cv_w[:, 0, 0, 0]
        out[i * 16:(i + 1) * 16, 2] = cv_b
    for g in range(8):
        out[6 * g:6 * g + 6, 3] = fm_b[1] - fm_b[0]      # sigmoid bias: mask
        out[64 + 6 * g:64 + 6 * g + 6, 3] = bd_b[1] - bd_b[0]  # boundary
    for r in range(8):
        out[r * 16:r * 16 + 16, 4] = fuse_b          # interior fuse bias
        out[r * 16:r * 16 + 16, 5] = fuse_b if r > 0 else 0.0   # first tile
        out[r * 16:r * 16 + 16, 6] = fuse_b if r < 3 else 0.0   # last tile
    return out


def _pack_inputs(xb, yb):
    """xp [NT,128,514] fp8; yhp [NT,40,512] fp8; ycp [NT,30,512] fp8."""
    B = xb.shape[0]
    x8 = xb.astype(F8)
    y8 = yb.astype(F8)
    xpad = np.zeros((B, 16, SB * NT + 8, W + 2), F8)
    xpad[:, :, 1:H + 1, 1:W + 1] = x8
    ridx = SB * np.arange(NT)[:, None] + np.arange(8)[None, :]
    xp = xpad[:, :, ridx, :].transpose(0, 2, 3, 1, 4).reshape(B, NT, 128, XW)
    ypad = np.zeros((B, 5, SB * NT + 8, W), F8)
    ypad[:, :, 1:H + 1, :] = y8
    yhp = ypad[:, :, ridx, :].transpose(0, 2, 3, 1, 4).reshape(B, NT, 40, W)
    cidx = SB * np.arange(NT)[:, None] + 1 + np.arange(SB)[None, :]
    ycp = ypad[:, :, cidx, :].transpose(0, 2, 3, 1, 4).reshape(B, NT, 30, W)
    return (np.ascontiguousarray(xp), np.ascontiguousarray(yhp),
            np.ascontiguousarray(ycp))


# ---------------------------------------------------------------------------
# bass graph
# ---------------------------------------------------------------------------

def _pairs(v, pair_stride, n, base=0):
    """3-dim DR rhs AP [[part],[pair_stride,2],[1,n]] from a 2-dim view."""
    a = v.unsqueeze(1).copy()
    a.ap[1] = [pair_stride, 2]
    a.ap[2] = [1, n]
    return a


def _build():
    import concourse.bacc as bacc
    import concourse.tile as tile
    from concourse import mybir

    f32 = mybir.dt.float32
    bf16 = mybir.dt.bfloat16
    f8 = mybir.dt.float8e4
    AF = mybir.ActivationFunctionType
    ALU = mybir.AluOpType
    DR = mybir.MatmulPerfMode.DoubleRow

    def _pick(act_cost, dve_cost):
        bal = _cache.setdefault("_bal", [0.0, 0.0])
        bal[0] *= BAL_DECAY
        bal[1] *= BAL_DECAY
        if (bal[0] + act_cost) * ACT_W <= bal[1] + dve_cost:
            bal[0] += act_cost
            return 0
        bal[1] += dve_cost
        return 1

    nc = bacc.Bacc("TRN2", target_bir_lowering=False)
    xp_ext = nc.declare_dram_parameter("xp", [NT, 128, XW], f8, isOutput=False)
    yhp_ext = nc.declare_dram_parameter("yhp", [NT, 40, W], f8, isOutput=False)
    ycp_ext = nc.declare_dram_parameter("ycp", [NT, 30, W], f8, isOutput=False)
    wc8_ext = nc.declare_dram_parameter("wc8", [128, W8TOT], f8, isOutput=False)
    wcb_ext = nc.declare_dram_parameter("wcb", [128, WBTOT], bf16,
                                        isOutput=False)
    fcol_ext = nc.declare_dram_parameter("fcol", [128, FCOLS], f32,
                                         isOutput=False)
    out_ext = nc.declare_dram_parameter("out", [NT, 96, W], bf16,
                                        isOutput=True)

    with tile.TileContext(nc) as tc:
        with (
            tc.tile_pool(name="singles", bufs=1) as singles,
            tc.tile_pool(name="ps_fuse", bufs=FUSE_BUFS, space="PSUM")
                as ps_fuse,
            tc.tile_pool(name="ps_conv", bufs=2, space="PSUM") as ps_conv,
            tc.tile_pool(name="ps_lc", bufs=LC_BUFS, space="PSUM") as ps_lc,
        ):
            # ---------------- static tiles + warmup -----------------------
            wtile = singles.tile([128, 256], bf16, tag="wtile")
            nc.vector.memset(wtile[:, :], 0.0)
            wps = ps_conv.tile([96, 1024], f32, tag="conv", name="warmps")
            for i in range(N_WARM):
                nc.tensor.matmul(wps[0:96, 0:256], lhsT=wtile[:, 0:96],
                                 rhs=wtile[:, :], start=(i == 0),
                                 stop=(i == N_WARM - 1))

            wc8 = singles.tile([128, W8TOT], f8, tag="wc8")
            nc.sync.dma_start(out=wc8[:, :], in_=wc8_ext[:, :])
            wcb = singles.tile([128, WBTOT], bf16, tag="wcb")
            fcol = singles.tile([128, FCOLS], f32, tag="fcol")
            nc.sync.dma_start(out=fcol[:, :], in_=fcol_ext[:, :])

            def conv_lhsT(d):
                a = wc8[:, 192 * d:192 * (d + 1)].unsqueeze(1).copy()
                a.ap[1] = [96, 2]
                a.ap[2] = [1, 96]
                return a

            def fuse_lhsT(v):
                a = wc8[0:40, C_FUSE + 256 * v:C_FUSE + 256 * (v + 1)] \
                    .unsqueeze(1).copy()
                a.ap[1] = [128, 2]
                a.ap[2] = [1, 128]
                return a

            def lbms_lhsT(g):
                a = wc8[0:126, C_LBMS + 256 * g:C_LBMS + 256 * (g + 1)] \
                    .unsqueeze(1).copy()
                a.ap[1] = [128, 2]
                a.ap[2] = [1, 128]
                return a

            SELW = wcb[:, B_SEL:B_SEL + 96]
            W1L = wcb[:, B_W1:B_W1 + 96]
            W2R = wcb[:, B_W2:B_W2 + 128]

            # ---------------- data tiles ----------------------------------
            xf = singles.tile([128, NT * PW], f8, tag="xf")
            yh = singles.tile([40, RING_YH * XW], f8, tag="yh")
            fcc = singles.tile([128, NT * FW], f8, tag="fcc")
            og = [singles.tile([96, 8 * W], bf16, tag=f"og{k}", name=f"og{k}")
                  for k in (0, 1, 2)]
            sg = [singles.tile([112, W], bf16, tag=f"sg{k}", name=f"sg{k}")
                  for k in (0, 1)]
            sgB = [singles.tile([48, W], bf16, tag=f"sgB{k}", name=f"sgB{k}")
                   for k in (0, 1)]
            svg = [singles.tile([49, W], bf16, tag=f"svg{k}", name=f"svg{k}")
                   for k in (0, 1, 2)]
            rep = [singles.tile([96, W], bf16, tag=f"rep{k}", name=f"rep{k}")
                   for k in (0, 1, 2, 3)]
            Ra = singles.tile([128, NT], f32, tag="Ra")
            nc.vector.memset(Ra[:, :], 0.0)
            Rb = singles.tile([128, NT], f32, tag="Rb")
            nc.vector.memset(Rb[:, :], 0.0)

            # presets: XF F-block pad cols (t=0,513 per ring slot); yh pad
            # cols 512/513 per slot; fcc pad cols 512/513 per slot.
            m = xf[:, XW:XW + 1].unsqueeze(1).copy()
            m.ap[1] = [PW, NT]
            m.ap[2] = [XW - 1, 2]
            nc.gpsimd.memset(m, 0.0)
            m = yh[:, W:W + 1].unsqueeze(1).copy()
            m.ap[1] = [XW, RING_YH]
            m.ap[2] = [1, 2]
            nc.gpsimd.memset(m, 0.0)
            m = fcc[:, W:W + 1].unsqueeze(1).copy()
            m.ap[1] = [FW, NT]
            m.ap[2] = [1, 2]
            nc.gpsimd.memset(m, 0.0)

            # ---------------- phase A + fronts ----------------------------
            def xslot(t):
                return xf[:, t * PW:t * PW + PW]

            def issue_fuse(t):
                if t % 8 == 0:
                    n = min(8, NT - t)
                    s1 = (t % RING_YH)
                    nc.sync.dma_start(
                        out=yh[:, s1 * XW:(s1 + n) * XW].rearrange(
                            "p (s c) -> p s c", s=n)[:, :, 0:W],
                        in_=yhp_ext[t:t + n, :, :].rearrange("s p j -> p s j"))
                    nc.sync.dma_start(
                        out=xf[:, t * PW:(t + n) * PW].rearrange(
                            "p (s c) -> p s c", s=n)[:, :, 0:XW],
                        in_=xp_ext[t:t + n, :, :].rearrange("s p j -> p s j"))
                yv = yh[0:40, (t % RING_YH) * XW:(t % RING_YH) * XW + W]
                fps = ps_fuse.tile([128, 512], f32, tag="fuse")
                v = 1 if t == 0 else (2 if t == NT - 1 else 0)
                nc.tensor.matmul(fps[:, :], lhsT=fuse_lhsT(v),
                                 rhs=_pairs(yv, 2, W), start=True, stop=True,
                                 perf_mode=DR)
                fdst = xslot(t)[:, XW + 1:XW + 1 + W]
                bv = 4 if 0 < t < NT - 1 else (5 if t == 0 else 6)
                bias = fcol[:, bv:bv + 1]
                if _pick(612, 658) == 0:
                    nc.scalar.activation(out=fdst, in_=fps[:, :], func=AF.Relu,
                                         bias=bias, accum_out=Ra[:, t:t + 1])
                else:
                    nc.vector.tensor_scalar(out=fdst, in0=fps[:, :],
                                            scalar1=bias, scalar2=0.0,
                                            op0=ALU.add, op1=ALU.max,
                                            accum_out=Rb[:, t:t + 1])

            cpair = {}

            def issue_front(s):
                xv = xslot(s)
                CD = CONV_DEPTH
                if s % CD == 0:
                    cpair[0] = ps_conv.tile([96, CD * 512], f32, tag="conv",
                                            name="cps")
                cps = cpair[0]
                half = (s % CD) * 512
                for d in range(3):
                    nc.tensor.matmul(cps[:, half:half + 512],
                                     lhsT=conv_lhsT(d),
                                     rhs=_pairs(xv[:, d:], XW, W),
                                     start=(d == 0), stop=(d == 2),
                                     perf_mode=DR)
                nb = s % CD + 1
                if s % CD != CD - 1 and s != NT - 1:
                    return
                s0 = s - nb + 1
                dst = fcc[0:96, s0 * FW:s0 * FW + 1].unsqueeze(1).copy()
                dst.ap[1] = [FW, nb]
                dst.ap[2] = [1, W]
                src = cps[:, 0:nb * 512]
                if _pick(185 + nb * 427, 125 + nb * 533) == 0:
                    nc.scalar.activation(out=dst, in_=src, func=AF.Relu,
                                         bias=fcol[0:96, 0:1])
                else:
                    nc.vector.tensor_scalar(out=dst, in0=src,
                                            scalar1=fcol[0:96, 0:1],
                                            scalar2=0.0,
                                            op0=ALU.add, op1=ALU.max)

            for t in range(NT):
                if t == 2:
                    nc.sync.dma_start(out=wcb[:, :], in_=wcb_ext[:, :])
                    for k in (0, 1, 2):
                        nc.sync.dma_start(
                            out=svg[k][48:49, :],
                            in_=wcb[48:49, B_ONES:B_ONES + 512])
                issue_fuse(t)

            # ---------------- SE chain ------------------------------------
            R_bf = singles.tile([128, NT], bf16, tag="Rbf")
            nc.vector.tensor_add(out=R_bf[:, :], in0=Ra[:, :], in1=Rb[:, :])
            gps = ps_fuse.tile([96, NT], f32, tag="fuse")
            nc.tensor.matmul(gps[:, :], lhsT=SELW, rhs=R_bf[:, :],
                             start=True, stop=True)
            gap_f = singles.tile([96, 1], f32, tag="gapf")
            nc.vector.reduce_sum(out=gap_f[:, :], in_=gps[:, :],
                                 axis=mybir.AxisListType.X)
            gap_bf = singles.tile([128, 1], bf16, tag="gap")
            nc.vector.memset(gap_bf[:, :], 0.0)
            nc.vector.tensor_copy(out=gap_bf[0:96, :], in_=gap_f[:, :])
            hps = ps_fuse.tile([96, 1], f32, tag="fuse")
            nc.tensor.matmul(hps[:, :], lhsT=W1L, rhs=gap_bf[:, :],
                             start=True, stop=True)
            h_bf = singles.tile([128, 1], bf16, tag="hbf")
            nc.vector.memset(h_bf[:, :], 0.0)
            nc.scalar.activation(out=h_bf[0:96, :], in_=hps[:, :], func=AF.Relu)
            sps = ps_fuse.tile([128, 1], f32, tag="fuse")
            nc.tensor.matmul(sps[:, :], lhsT=W2R, rhs=h_bf[:, :],
                             start=True, stop=True)
            se_bc = singles.tile([128, 1], f32, tag="sebc")
            nc.scalar.activation(out=se_bc[:, :], in_=sps[:, :],
                                 func=AF.Sigmoid)
            # fill LBMS bd rows (96..125): wc8 <- PSBW * se
            nc.scalar.activation(out=wc8[96:126, C_LBMS:C_LBMS + 1024],
                                 in_=wcb[96:126, B_PSBW:B_PSBW + 1024],
                                 func=AF.Copy, scale=se_bc[96:126, :])
            nc.vector.tensor_scalar(
                out=wc8[96:126, C_LBMS + 1024:C_LBMS + 2048],
                in0=wcb[96:126, B_PSBW + 1024:B_PSBW + 2048],
                scalar1=se_bc[96:126, :], scalar2=0.0,
                op0=ALU.mult, op1=ALU.add)

            # ---------------- tails ---------------------------------------
            def issue_tail(u):
                g = u % 8
                G = u // 8
                gp = _cache.setdefault("_gp", {})
                if g == 0:
                    gp[G] = ps_fuse.tile([128, 512], f32, tag="fuse",
                                         name="grp")
                GP = gp[G]
                fv = fcc[0:126, u * FW:u * FW + W]
                nc.tensor.matmul(GP[:, :], lhsT=lbms_lhsT(g),
                                 rhs=_pairs(fv, 2, W), start=(g == 0),
                                 stop=(g == 7 or u == NT - 1), perf_mode=DR)
            def issue_chain(G):
                u = min(8 * G + 7, NT - 1)
                ng = u - 8 * G + 1           # strips in this group
                gp = _cache.setdefault("_gp", {})
                GP = gp[G]
                sgt, svt = sg[G % 2], svg[G % 3]
                np_ = 64 + 6 * ng
                bal = _cache.setdefault("_bal", [0.0, 0.0])
                bal[0] += 612.0
                nc.scalar.activation(out=sgt[0:np_, :], in_=GP[0:np_, :],
                                     func=AF.Sigmoid, bias=fcol[0:np_, 3:4])
                sbt = sgB[G % 2]
                ce = nc.gpsimd if CHAIN_POOL else nc.vector
                if not CHAIN_POOL:
                    bal[1] += 713.0
                ce.tensor_copy(out=sbt[0:6 * ng, :],
                               in_=sgt[64:64 + 6 * ng, :])
                ce.tensor_add(out=svt[0:6 * ng, :],
                              in0=sgt[0:6 * ng, :],
                              in1=sbt[0:6 * ng, :])
                ce.tensor_scalar(out=svt[0:6 * ng, :],
                                 in0=svt[0:6 * ng, :],
                                 scalar1=1.0, scalar2=0.0,
                                 op0=ALU.min, op1=ALU.add)

            def og_dma(G, half=None):
                ng = min(8, NT - 8 * G)
                lo, hi = 0, ng
                if half == 0:
                    hi = min(4, ng)
                elif half == 1:
                    lo = 4
                    if ng <= 4:
                        return
                nc.sync.dma_start(
                    out=out_ext[8 * G + lo:8 * G + hi, :, :].rearrange(
                        "s p j -> p s j"),
                    in_=og[G % 3][0:96, lo * W:hi * W])

            def issue_final(uu):
                G = uu // 8
                gg = uu % 8
                svt = svg[G % 3]
                if True:
                    dst = og[G % 3][:, gg * W:gg * W + W]
                    if _is_rep(uu):
                        rt = rep[_cache.setdefault('_rr', [0])[0] % 4]
                        _cache['_rr'][0] += 1
                        src = svt[6 * gg:6 * gg + 6, :].unsqueeze(1).copy()
                        src.ap[1] = [0, 16]
                        nc.sync.dma_start(out=rt[:, :], in_=src)
                        k = _cache.setdefault("_repn", [0])
                        k[0] += 1
                        eng = nc.gpsimd if k[0] % AFF_MOD != 0 else nc.vector
                        eng.tensor_scalar(out=dst, in0=rt[:, :],
                                          scalar1=fcol[0:96, 1:2],
                                          scalar2=fcol[0:96, 2:3],
                                          op0=ALU.mult, op1=ALU.add)
                    else:
                        ops = ps_lc.tile([96, 512], f32, tag="lc")
                        nc.tensor.matmul(ops[:, :],
                                         lhsT=wcb[0:49, B_LCW + 96 * gg:
                                                  B_LCW + 96 * (gg + 1)],
                                         rhs=svt[0:49, :],
                                         start=True, stop=True)
                        if _pick(612, 658) == 0:
                            nc.scalar.activation(out=dst, in_=ops[:, :],
                                                 func=AF.Copy)
                        else:
                            nc.vector.tensor_copy(out=dst, in_=ops[:, :])

            for s0 in range(0, NT, 8):
                n = min(8, NT - s0)
                nc.sync.dma_start(
                    out=fcc[96:126, s0 * FW:(s0 + n) * FW].rearrange(
                        "p (s c) -> p s c", s=n)[:, :, 0:W],
                    in_=ycp_ext[s0:s0 + n, :, :].rearrange("s p j -> p s j"))
            tc = [0]
            fc_ = [0]
            cc = [0]
            vu = [-1]
            s = 0
            while fc_[0] < NT:
                if s < NT:
                    issue_front(s)
                pace = 1 if s < NT else 2
                if s >= TAIL_LAG:
                    vu[0] += pace
                for _ in range(pace):
                    if tc[0] < NT and tc[0] <= vu[0]:
                        issue_tail(tc[0])
                        tc[0] += 1
                while cc[0] < NG and vu[0] - min(8 * cc[0] + 7, NT - 1) \
                        >= CHAIN_LAG:
                    issue_chain(cc[0])
                    cc[0] += 1
                uf_max = vu[0] - CHAIN_LAG - FIN_LAG
                nfin = pace
                while fc_[0] <= uf_max and fc_[0] < NT and nfin > 0 \
                        and fc_[0] // 8 < cc[0]:
                    uf = fc_[0]
                    fc_[0] += 1
                    nfin -= 1
                    issue_final(uf)
                    if uf // 8 == NG - 1:
                        if uf % 8 == 3:
                            og_dma(uf // 8, 0)
                        elif uf == NT - 1:
                            og_dma(uf // 8, 1)
                    elif uf % 8 == 7:
                        og_dma(uf // 8)
                s += 1
    nc.compile()
    for k in ("_gp", "_lcn", "_repn", "_bal", "_rr", "_rq", "_fc", "_cc"):
        _cache.pop(k, None)
    return nc


# ---------------------------------------------------------------------------
# entry point
# ---------------------------------------------------------------------------

LAST_RESULT = None


def prepare(x, y, fuse_w, fuse_b, se_w1, se_w2, bd_w, bd_b,
            fc_w, fc_b, fm_w, fm_b, cv_w, cv_b):
    if "nc" not in _cache:
        _cache["nc"] = _build()
    nc = _cache["nc"]
    g = {k: np.asarray(v, np.float32) for k, v in dict(
        fuse_w=fuse_w, fuse_b=fuse_b, se_w1=se_w1, se_w2=se_w2, bd_w=bd_w,
        bd_b=bd_b, fc_w=fc_w, fc_b=fc_b, fm_w=fm_w, fm_b=fm_b, cv_w=cv_w,
        cv_b=cv_b).items()}
    wc8 = np.zeros((128, W8TOT), F8)
    wc8[:, 0:576] = _conv_w8(g["fc_w"])
    wc8[0:40, C_FUSE:C_FUSE + 768] = _fuse_w8(g["fuse_w"])
    wc8[:, C_LBMS:C_LBMS + 2048] = _lbms_w8(g["fm_w"])
    wcb = _wcb(g["cv_w"], g["cv_b"], g["se_w1"], g["se_w2"], g["bd_w"])
    fcol = _fcol(g["fc_b"], g["cv_w"], g["cv_b"], g["fm_b"], g["bd_b"],
                 g["fuse_b"])
    xb = np.asarray(x, np.float32)
    yb = np.asarray(y, np.float32)
    xp, yhp, ycp = _pack_inputs(xb, yb)
    in_maps = [
        {"xp": xp[i], "yhp": yhp[i], "ycp": ycp[i],
         "wc8": wc8, "wcb": wcb, "fcol": fcol}
        for i in range(xb.shape[0])
    ]
    return nc, in_maps


def kernel(x, y, fuse_w, fuse_b, se_w1, se_w2, bd_w, bd_b,
           fc_w, fc_b, fm_w, fm_b, cv_w, cv_b):
    global LAST_RESULT
    from concourse.bass_utils import run_bass_kernel_spmd

    nc, in_maps = prepare(x, y, fuse_w, fuse_b, se_w1, se_w2, bd_w, bd_b,
                          fc_w, fc_b, fm_w, fm_b, cv_w, cv_b)
    res = run_bass_kernel_spmd(nc, in_maps, core_ids=list(range(8)))
    LAST_RESULT = res
    outs = []
    for i in range(len(in_maps)):
        ot = np.asarray(res.results[i]["out"], np.float32)   # [NT, 96, W]
        full = ot.reshape(NT, SB, 16, W).transpose(2, 0, 1, 3) \
                 .reshape(16, NT * SB, W)[:, :H, :]
        outs.append(full)
    return np.stack(outs)
